# revision 1
# baseline (speedup 1.0000x reference)
"""BiLSTM tagger on 8 TRN2 NeuronCores.

Strategy (hardcoded for B=64,T=512,V=30000,E=128,H=256,TAGS=50):
  - Data-parallel: batch sharded 8 ways (8 sequences/core); weights replicated.
  - Per core: embedding gather (indirect DMA) -> PE transpose -> x^T in SBUF;
    input projections xg = W_ih_aug @ [x; 1-m; 1] precomputed for all t as big
    matmuls into DRAM scratch; recurrences (l1 fwd+bwd interleaved, then l2
    fwd+bwd) as dynamic Tile loops, 16 steps per iteration; classifier matmul.
  - Masking: the (1-m) feature adds +/-60 to the f/i gate pre-activations at
    masked steps, freezing c exactly (sigmoid saturates to 1.0/0.0 in fp32).
    Backward-direction h is then exactly 0 at masked steps. Forward l2 output
    h is repaired with a per-step output-side blend using m / (1-m) planes
    carried in the xg stream. l1f's garbage masked outputs only feed masked
    l2 steps, where c is frozen anyway.
  - Gate layout: gates on partitions (8 chunks of 128 = [i0 i1 f0 f1 o0 o1 g0 g1]),
    batch on free dim -> cheap pointwise; Whh stationary [128h x 128gate] bf16
    tiles (FWL), h moving [128, 8].
"""
import sys

sys.path.insert(0, "/opt/trn_rl_repo")
import contextlib

import numpy as np
import ml_dtypes

import concourse.bass as bass
import concourse.bacc as bacc
import concourse.mybir as mybir
import concourse.tile as tile
from concourse.bass import ds
from concourse.bass_utils import run_bass_kernel_spmd
from concourse.masks import make_identity

B, T, V, E, H, TAGS = 64, 512, 30000, 128, 256, 50
NCORES = 8
Bc = B // NCORES          # 8 sequences per core
TB = T * Bc               # 4096 tokens per core
STEPS_PER_BODY = 16
NBODY = T // STEPS_PER_BODY  # 32

f32 = mybir.dt.float32
bf16 = mybir.dt.bfloat16
i32 = mybir.dt.int32

UNITS = ("1f", "1b", "2f", "2b")
KCNT = {"1f": 1, "1b": 1, "2f": 4, "2b": 4}       # 128-row K chunks of x features
MCNT = {"1f": 8, "1b": 8, "2f": 12, "2b": 8}      # 128-row output chunks
REV = {"1f": False, "1b": True, "2f": False, "2b": True}

# gate chunk order i0 i1 f0 f1 o0 o1 g0 g1 (torch row order is i f g o)
PERM = np.concatenate([np.arange(0, 256), np.arange(256, 512),
                       np.arange(768, 1024), np.arange(512, 768)])

_CACHE = {}


def _prep_unit_weights(Wih, Whh, bih, bhh, m_cnt):
    """Host-side weight marshalling for one LSTM direction."""
    din = Wih.shape[1]
    Wp = np.asarray(Wih)[PERM]          # [1024, din]
    Up = np.asarray(Whh)[PERM]          # [1024, 256]
    bp = (np.asarray(bih) + np.asarray(bhh))[PERM]  # [1024]
    M = m_cnt * 128
    k_cnt = din // 128
    # x-part lhsT: [din, M] -> k-chunk-major cols [128, k_cnt*M]
    WT = np.zeros((din, M), np.float32)
    WT[:, :1024] = Wp.T
    wx = np.concatenate([WT[k * 128:(k + 1) * 128, :] for k in range(k_cnt)],
                        axis=1).astype(ml_dtypes.bfloat16)  # [128, k_cnt*M]
    # aug lhsT rows: feature0 = (1-m), feature1 = 1
    wa = np.zeros((2, M), np.float32)
    wa[0, 0:256] = -60.0   # i rows: -60*(1-m)
    wa[0, 256:512] = 60.0  # f rows: +60*(1-m)
    wa[1, :1024] = bp
    if m_cnt == 12:        # l2f extra planes: m, m, 1-m, 1-m
        wa[0, 1024:1280] = -1.0
        wa[1, 1024:1280] = 1.0
        wa[0, 1280:1536] = 1.0
    wa = wa.astype(ml_dtypes.bfloat16)
    # Whh lhsT: [256, 1024] -> [128, 2*1024]
    UT = Up.T.astype(np.float32)
    wh = np.concatenate([UT[0:128, :], UT[128:256, :]], axis=1).astype(ml_dtypes.bfloat16)
    return wx, wa, wh


def _build_program(stage="full", repeat=1):
    nc = bacc.Bacc("TRN2", target_bir_lowering=False, debug=False, num_devices=NCORES)
    emb_d = nc.dram_tensor("emb", [V, E], f32, kind="ExternalInput")
    words_d = nc.dram_tensor("words", [TB, 1], i32, kind="ExternalInput")
    aug_d = nc.dram_tensor("aug", [2, TB], bf16, kind="ExternalInput")
    wxd, wad, whd, xgd = {}, {}, {}, {}
    for u in UNITS:
        wxd[u] = nc.dram_tensor(f"w{u}x", [128, KCNT[u] * MCNT[u] * 128], bf16, kind="ExternalInput")
        wad[u] = nc.dram_tensor(f"w{u}a", [2, MCNT[u] * 128], bf16, kind="ExternalInput")
        whd[u] = nc.dram_tensor(f"w{u}h", [128, 2048], bf16, kind="ExternalInput")
        xgd[u] = nc.dram_tensor(f"xg{u}", [MCNT[u], 128, TB], f32)
    clsx_d = nc.dram_tensor("clsx", [128, 4 * TAGS], bf16, kind="ExternalInput")
    clsb_d = nc.dram_tensor("clsb", [TAGS, 1], f32, kind="ExternalInput")
    logits_d = nc.dram_tensor("logits", [TAGS, TB], f32, kind="ExternalOutput")

    ctx = contextlib.ExitStack()
    with tile.TileContext(nc) as tc, ctx:
        pp = ctx.enter_context(tc.tile_pool(name="persist", bufs=1))
        xT = pp.tile([128, TB], bf16, tag="xT")
        aug_sb = pp.tile([2, TB], bf16, tag="aug")
        ident = pp.tile([128, 128], f32, tag="ident")
        wx_sb = {u: pp.tile([128, KCNT[u] * MCNT[u] * 128], bf16, tag=f"wx{u}", name=f"wx{u}") for u in UNITS}
        wa_sb = {u: pp.tile([2, MCNT[u] * 128], bf16, tag=f"wa{u}", name=f"wa{u}") for u in UNITS}
        wh_sb = {u: pp.tile([128, 2048], bf16, tag=f"wh{u}", name=f"wh{u}") for u in UNITS}
        cls_sb = pp.tile([128, 4 * TAGS], bf16, tag="clsx")
        clsb_sb = pp.tile([TAGS, 1], f32, tag="clsb")
        hs = {u: pp.tile([128, T, 2, Bc], bf16, tag=f"hs{u}", name=f"hs{u}") for u in UNITS}
        o2f_sb = pp.tile([128, T, 2, Bc], bf16, tag="o2f")
        hcar = {u: pp.tile([128, 2, Bc], bf16, tag=f"hc{u}", name=f"hc{u}") for u in UNITS}
        ccar = {u: pp.tile([128, 2, Bc], f32, tag=f"cc{u}", name=f"cc{u}") for u in UNITS}
        o2f_car = pp.tile([128, 2, Bc], bf16, tag="o2fc")

        # ---- load weights / constants
        make_identity(nc, ident[:])
        for u in UNITS:
            nc.sync.dma_start(wx_sb[u][:], wxd[u][:])
            nc.sync.dma_start(wa_sb[u][:], wad[u][:])
            nc.sync.dma_start(wh_sb[u][:], whd[u][:])
        nc.sync.dma_start(cls_sb[:], clsx_d[:])
        nc.sync.dma_start(clsb_sb[:], clsb_d[:])
        nc.sync.dma_start(aug_sb[:], aug_d[:])
        for u in UNITS:
            nc.vector.memset(hcar[u][:, :, :], 0.0)
            nc.vector.memset(ccar[u][:, :, :], 0.0)
        nc.vector.memset(o2f_car[:, :, :], 0.0)

        # ---- embedding gather + transpose into xT
        for _rep in range(repeat):
         with tc.tile_pool(name=f"gat{_rep}", bufs=3) as gp, \
             tc.tile_pool(name=f"gps{_rep}", bufs=3, space="PSUM") as gps:
            for n in range(TB // 128):
                idx = gp.tile([128, 1], i32, tag="idx")
                nc.sync.dma_start(idx[:], words_d[n * 128:(n + 1) * 128, :])
                xt = gp.tile([128, 128], f32, tag="xt")
                nc.gpsimd.indirect_dma_start(
                    out=xt[:], out_offset=None, in_=emb_d[:, :],
                    in_offset=bass.IndirectOffsetOnAxis(ap=idx[:, :1], axis=0))
                pst = gps.tile([128, 128], f32, tag="pst")
                nc.tensor.transpose(out=pst[:], in_=xt[:], identity=ident[:])
                nc.vector.tensor_copy(xT[:, n * 128:(n + 1) * 128], pst[:])

        # ---- xg precompute
        def xg_precompute(u, rhs_of_k, _rep=0):
            m_cnt, k_cnt = MCNT[u], KCNT[u]
            with tc.tile_pool(name=f"xp{u}{_rep}", bufs=4, space="PSUM") as xps, \
                 tc.tile_pool(name=f"xs{u}{_rep}", bufs=4) as xsb:
                for n in range(TB // 512):
                    nsl = slice(n * 512, (n + 1) * 512)
                    for m in range(m_cnt):
                        psm = xps.tile([128, 512], f32, tag="ps")
                        first = True
                        if m < 8:  # gate chunks get the x contribution
                            for k in range(k_cnt):
                                nc.tensor.matmul(
                                    out=psm[:],
                                    lhsT=wx_sb[u][:, (k * m_cnt + m) * 128:(k * m_cnt + m + 1) * 128],
                                    rhs=rhs_of_k(k, n),
                                    start=first, stop=False)
                                first = False
                        nc.tensor.matmul(
                            out=psm[:],
                            lhsT=wa_sb[u][:, m * 128:(m + 1) * 128],
                            rhs=aug_sb[:, nsl],
                            start=first, stop=True)
                        stg = xsb.tile([128, 512], f32, tag="stg")
                        if (n + m) % 2 == 0:
                            nc.vector.tensor_copy(stg[:], psm[:])
                        else:
                            nc.scalar.activation(stg[:], psm[:],
                                                 mybir.ActivationFunctionType.Copy)
                        nc.sync.dma_start(xgd[u][m, :, nsl], stg[:])

        def l1_rhs(k, n):
            return xT[:, n * 512:(n + 1) * 512]

        if stage != "gather":
            for _rep in range(repeat):
                xg_precompute("1f", l1_rhs, _rep)
                xg_precompute("1b", l1_rhs, _rep)

        # ---- recurrence phase
        def phase(units, _rep=0):
            with tc.tile_pool(name=f"rc{units[0]}{_rep}", bufs=2) as rp, \
                 tc.tile_pool(name=f"rps{units[0]}{_rep}", bufs=4, space="PSUM") as rps, \
                 tc.tile_pool(name=f"rtmp{units[0]}{_rep}", bufs=3) as tp:
                with tc.For_i(0, NBODY, hint_engines=(mybir.EngineType.PE,)) as i:
                    for u in units:
                        m_cnt = MCNT[u]
                        rev = REV[u]
                        xb = rp.tile([128, m_cnt, 128], f32, tag=f"xb{u}")
                        if rev:
                            col0 = i * (-128) + (TB - 128)
                            t0 = i * (-STEPS_PER_BODY) + (T - STEPS_PER_BODY)
                        else:
                            col0 = i * 128
                            t0 = i * STEPS_PER_BODY
                        for m in range(m_cnt):
                            nc.sync.dma_start(xb[:, m, :], xgd[u][m, :, ds(col0, 128)])
                        hstage = rp.tile([128, STEPS_PER_BODY, 2, Bc], bf16, tag=f"hst{u}")
                        if u == "2f":
                            ostage = rp.tile([128, STEPS_PER_BODY, 2, Bc], bf16, tag="ost")
                        for us in range(STEPS_PER_BODY):
                            slot = (STEPS_PER_BODY - 1 - us) if rev else us
                            bc = slot * Bc
                            if us == 0:
                                hprev = hcar[u]
                            else:
                                pslot = slot + 1 if rev else slot - 1
                                hprev = hstage[:, pslot, :, :]
                            psm = rps.tile([128, 8, Bc], f32, tag="g")
                            for m in range(8):
                                for k in range(2):
                                    nc.tensor.matmul(
                                        out=psm[:, m, :],
                                        lhsT=wh_sb[u][:, (k * 8 + m) * 128:(k * 8 + m + 1) * 128],
                                        rhs=hprev[:, k, :] if us else hprev[:, k, :],
                                        start=(k == 0), stop=(k == 1))
                            g = tp.tile([128, 8, Bc], f32, tag="gs")
                            nc.vector.tensor_tensor(
                                out=g[:, :, :], in0=psm[:, :, :],
                                in1=xb[:, 0:8, bc:bc + Bc], op=mybir.AluOpType.add)
                            sg = tp.tile([128, 8, Bc], f32, tag="sg")
                            nc.scalar.activation(sg[:, 0:6, :], g[:, 0:6, :],
                                                 mybir.ActivationFunctionType.Sigmoid)
                            nc.scalar.activation(sg[:, 6:8, :], g[:, 6:8, :],
                                                 mybir.ActivationFunctionType.Tanh)
                            t1 = tp.tile([128, 2, Bc], f32, tag="t1")
                            nc.vector.tensor_tensor(out=t1[:, :, :], in0=sg[:, 0:2, :],
                                                    in1=sg[:, 6:8, :], op=mybir.AluOpType.mult)
                            csf = tp.tile([128, 2, Bc], f32, tag="csf")
                            nc.vector.tensor_tensor(out=csf[:, :, :], in0=sg[:, 2:4, :],
                                                    in1=ccar[u][:, :, :], op=mybir.AluOpType.mult)
                            nc.vector.tensor_tensor(out=ccar[u][:, :, :], in0=csf[:, :, :],
                                                    in1=t1[:, :, :], op=mybir.AluOpType.add)
                            tc2 = tp.tile([128, 2, Bc], f32, tag="tc2")
                            nc.scalar.activation(tc2[:, :, :], ccar[u][:, :, :],
                                                 mybir.ActivationFunctionType.Tanh)
                            nc.vector.tensor_tensor(out=hstage[:, slot, :, :], in0=sg[:, 4:6, :],
                                                    in1=tc2[:, :, :], op=mybir.AluOpType.mult)
                            if u == "2f":
                                ma = tp.tile([128, 2, Bc], f32, tag="ma")
                                nc.vector.tensor_tensor(out=ma[:, :, :], in0=hstage[:, slot, :, :],
                                                        in1=xb[:, 8:10, bc:bc + Bc],
                                                        op=mybir.AluOpType.mult)
                                oprev = o2f_car if us == 0 else ostage[:, slot - 1, :, :]
                                mb = tp.tile([128, 2, Bc], f32, tag="mb")
                                nc.vector.tensor_tensor(out=mb[:, :, :], in0=oprev[:, :, :],
                                                        in1=xb[:, 10:12, bc:bc + Bc],
                                                        op=mybir.AluOpType.mult)
                                nc.vector.tensor_tensor(out=ostage[:, slot, :, :], in0=ma[:, :, :],
                                                        in1=mb[:, :, :], op=mybir.AluOpType.add)
                        # flush staged h history
                        nc.vector.tensor_copy(hs[u][:, ds(t0, STEPS_PER_BODY), :, :],
                                              hstage[:, :, :, :])
                        last_slot = 0 if rev else STEPS_PER_BODY - 1
                        nc.vector.tensor_copy(hcar[u][:, :, :], hstage[:, last_slot, :, :])
                        if u == "2f":
                            nc.vector.tensor_copy(o2f_sb[:, ds(t0, STEPS_PER_BODY), :, :],
                                                  ostage[:, :, :, :])
                            nc.vector.tensor_copy(o2f_car[:, :, :],
                                                  ostage[:, STEPS_PER_BODY - 1, :, :])

        if stage in ("ph1", "xg2", "full"):
            for _rep in range(repeat):
                phase(("1f", "1b"), _rep)

        def l2_rhs(k, n):
            src = hs["1f"] if k < 2 else hs["1b"]
            return src[:, n * 64:(n + 1) * 64, k % 2, :]

        if stage in ("xg2", "full"):
            for _rep in range(repeat):
                xg_precompute("2f", l2_rhs, _rep)
                xg_precompute("2b", l2_rhs, _rep)

        if stage == "full":
            for _rep in range(repeat):
                phase(("2f", "2b"), _rep)

        # ---- classifier
        if stage != "full":
            with tc.tile_pool(name="dum", bufs=1) as dp:
                dmy = dp.tile([TAGS, 512], f32, tag="dmy")
                nc.vector.memset(dmy[:], 0.0)
                nc.sync.dma_start(logits_d[:, 0:512], dmy[:])
        else:
         with tc.tile_pool(name="cl", bufs=3) as cp, \
             tc.tile_pool(name="cps", bufs=3, space="PSUM") as cps:
            for n in range(TB // 512):
                psm = cps.tile([TAGS, 512], f32, tag="ps")
                for k in range(4):
                    src = o2f_sb if k < 2 else hs["2b"]
                    nc.tensor.matmul(
                        out=psm[:],
                        lhsT=cls_sb[:, k * TAGS:(k + 1) * TAGS],
                        rhs=src[:, n * 64:(n + 1) * 64, k % 2, :],
                        start=(k == 0), stop=(k == 3))
                lg = cp.tile([TAGS, 512], f32, tag="lg")
                nc.vector.tensor_scalar_add(lg[:], psm[:], clsb_sb[:, :1])
                nc.sync.dma_start(logits_d[:, n * 512:(n + 1) * 512], lg[:])

    nc.compile()
    return nc


def _make_in_maps(inputs):
    words = np.asarray(inputs["words"]).astype(np.int32)
    lengths = np.asarray(inputs["lengths"]).astype(np.int32)
    emb = np.asarray(inputs["emb"], dtype=np.float32)
    mask = (lengths[:, None] > np.arange(T)[None, :]).astype(np.float32)
    wprep = {u: _prep_unit_weights(inputs[f"l{u}_Wih"], inputs[f"l{u}_Whh"],
                                   inputs[f"l{u}_bih"], inputs[f"l{u}_bhh"], MCNT[u])
             for u in UNITS}
    clsW = np.asarray(inputs["cls_W"], dtype=np.float32)
    CT = clsW.T
    clsx = np.concatenate([CT[k * 128:(k + 1) * 128, :] for k in range(4)],
                          axis=1).astype(ml_dtypes.bfloat16)
    clsb = np.asarray(inputs["cls_b"], dtype=np.float32).reshape(TAGS, 1)
    in_maps = []
    for c in range(NCORES):
        bsl = slice(c * Bc, (c + 1) * Bc)
        w_c = words[bsl]
        m_c = mask[bsl]
        words_tm = np.ascontiguousarray(w_c.T).reshape(TB, 1)
        aug = np.stack([(1.0 - m_c.T).reshape(TB), np.ones(TB, np.float32)]
                       ).astype(ml_dtypes.bfloat16)
        im = {"emb": emb, "words": words_tm, "aug": aug,
              "clsx": clsx, "clsb": clsb}
        for u in UNITS:
            wx, wa, wh = wprep[u]
            im[f"w{u}x"] = wx
            im[f"w{u}a"] = wa
            im[f"w{u}h"] = wh
        in_maps.append(im)
    return in_maps


def kernel(**inputs):
    words = np.asarray(inputs["words"]).astype(np.int32)      # [B, T]
    lengths = np.asarray(inputs["lengths"]).astype(np.int32)  # [B]
    emb = np.asarray(inputs["emb"], dtype=np.float32)

    if "nc" not in _CACHE:
        _CACHE["nc"] = _build_program()
    nc = _CACHE["nc"]

    mask = (lengths[:, None] > np.arange(T)[None, :]).astype(np.float32)  # [B,T]
    wprep = {u: _prep_unit_weights(inputs[f"l{u}_Wih"], inputs[f"l{u}_Whh"],
                                   inputs[f"l{u}_bih"], inputs[f"l{u}_bhh"], MCNT[u])
             for u in UNITS}
    clsW = np.asarray(inputs["cls_W"], dtype=np.float32)      # [50, 512]
    CT = clsW.T  # [512, 50]
    clsx = np.concatenate([CT[k * 128:(k + 1) * 128, :] for k in range(4)],
                          axis=1).astype(ml_dtypes.bfloat16)  # [128, 200]
    clsb = np.asarray(inputs["cls_b"], dtype=np.float32).reshape(TAGS, 1)

    in_maps = []
    for c in range(NCORES):
        bsl = slice(c * Bc, (c + 1) * Bc)
        w_c = words[bsl]                      # [Bc, T]
        m_c = mask[bsl]                       # [Bc, T]
        words_tm = np.ascontiguousarray(w_c.T).reshape(TB, 1)
        aug = np.stack([(1.0 - m_c.T).reshape(TB), np.ones(TB, np.float32)]
                       ).astype(ml_dtypes.bfloat16)           # [2, TB]
        im = {"emb": emb, "words": words_tm, "aug": aug,
              "clsx": clsx, "clsb": clsb}
        for u in UNITS:
            wx, wa, wh = wprep[u]
            im[f"w{u}x"] = wx
            im[f"w{u}a"] = wa
            im[f"w{u}h"] = wh
        in_maps.append(im)

    _CACHE["in_maps"] = in_maps
    res = run_bass_kernel_spmd(nc, in_maps, list(range(NCORES)))
    out = np.empty((B, T, TAGS), np.float32)
    for c in range(NCORES):
        lg = res.results[c]["logits"]          # [50, TB], col = t*Bc + b
        out[c * Bc:(c + 1) * Bc] = lg.reshape(TAGS, T, Bc).transpose(2, 1, 0)
    return out


def bench(inputs):
    """Run once with NTFF tracing; returns HW exec_time_ns (and stashes trace)."""
    kernel(**inputs)  # ensure program built/cached
    nc = _CACHE["nc"]
    in_maps = _CACHE["in_maps"]
    import tempfile
    tmpdir = tempfile.mkdtemp(prefix="bilstm_trace_")
    res = run_bass_kernel_spmd(nc, in_maps, list(range(NCORES)), trace=True,
                               tmpdir=tmpdir)
    _CACHE["trace_dir"] = tmpdir
    _CACHE["last_bench"] = res
    print("trace dir:", tmpdir)
    return res.exec_time_ns


if __name__ == "__main__":
    import reference
    inputs = {k: np.asarray(v) for k, v in reference.setup_inputs().items()}
    got = kernel(**inputs)
    print(got.shape, got.dtype)



# revision 3
# speedup vs baseline: 1.5100x; 1.5100x over previous
"""BiLSTM tagger on 8 TRN2 NeuronCores.

Strategy (hardcoded for B=64,T=512,V=30000,E=128,H=256,TAGS=50):
  - Data-parallel: batch sharded 8 ways (8 sequences/core); weights replicated.
  - Per core: embedding gather (indirect DMA) -> PE transpose -> x^T in SBUF;
    input projections xg = W_ih_aug @ [x; 1-m; 1] precomputed for all t as big
    matmuls into DRAM scratch; recurrences as dynamic Tile loops with the two
    directions of a layer FUSED: one pointwise instruction covers both units
    ([128, 2u, chunk, Bc] APs), halving DVE/Act instruction count and keeping
    a single dependency chain.
  - Masking: (1-m) feature adds +/-60 to f/i gate pre-activations at masked
    steps, freezing c exactly. Backward h is then exactly 0 at masked steps.
    Forward l2 h at masked steps is repaired AFTER the recurrence with a
    single tensor_tensor_scan per (hchunk, batch): hold[t] = (1-m)*hold[t-1]
    + m*h[t] (the LSTM "carry last valid h" rule), instead of per-step blends.
  - Gate layout: gates on partitions (8 chunks of 128 = [i0 i1 f0 f1 o0 o1 g0
    g1]), batch on free dim; Whh stationary [128h x 128gate] bf16 tiles, h
    moving [128, 8]. Reverse-direction steps share the fused ops via custom
    APs whose unit-dim stride jumps to the mirrored column (no per-unit ops).
"""
import sys

sys.path.insert(0, "/opt/trn_rl_repo")
import contextlib

import numpy as np
import ml_dtypes

import concourse.bass as bass
import concourse.bacc as bacc
import concourse.mybir as mybir
import concourse.tile as tile
from concourse.ap import AP
from concourse.bass import ds
from concourse.bass_utils import run_bass_kernel_spmd
from concourse.masks import make_identity

B, T, V, E, H, TAGS = 64, 512, 30000, 128, 256, 50
NCORES = 8
Bc = B // NCORES          # 8 sequences per core
TB = T * Bc               # 4096 tokens per core
SPB = 16                  # steps per For_i body
NBODY = T // SPB          # 32

f32 = mybir.dt.float32
bf16 = mybir.dt.bfloat16
i32 = mybir.dt.int32

UNITS = ("1f", "1b", "2f", "2b")
KCNT = {"1f": 1, "1b": 1, "2f": 4, "2b": 4}       # 128-row K chunks of x features
REV = {"1f": False, "1b": True, "2f": False, "2b": True}

# gate chunk order i0 i1 f0 f1 o0 o1 g0 g1 (torch row order is i f g o)
PERM = np.concatenate([np.arange(0, 256), np.arange(256, 512),
                       np.arange(768, 1024), np.arange(512, 768)])

_CACHE = {}


def _prep_unit_weights(Wih, Whh, bih, bhh):
    """Host-side weight marshalling for one LSTM direction (8 gate chunks)."""
    din = Wih.shape[1]
    Wp = np.asarray(Wih)[PERM]          # [1024, din]
    Up = np.asarray(Whh)[PERM]          # [1024, 256]
    bp = (np.asarray(bih) + np.asarray(bhh))[PERM]  # [1024]
    M = 1024
    k_cnt = din // 128
    # x-part lhsT: [din, 1024] -> k-chunk-major cols [128, k_cnt*1024]
    WT = Wp.T.astype(np.float32)        # [din, 1024]
    wx = np.concatenate([WT[k * 128:(k + 1) * 128, :] for k in range(k_cnt)],
                        axis=1).astype(ml_dtypes.bfloat16)  # [128, k_cnt*1024]
    # aug lhsT rows: feature0 = (1-m), feature1 = 1
    wa = np.zeros((2, M), np.float32)
    wa[0, 0:256] = -60.0   # i rows: -60*(1-m)
    wa[0, 256:512] = 60.0  # f rows: +60*(1-m)
    wa[1, :] = bp
    wa = wa.astype(ml_dtypes.bfloat16)
    # Whh lhsT: [256, 1024] -> [128, 2*1024]
    UT = Up.T.astype(np.float32)
    wh = np.concatenate([UT[0:128, :], UT[128:256, :]], axis=1).astype(ml_dtypes.bfloat16)
    return wx, wa, wh


def _uap(base, us, ustride_delta, step):
    """AP over `base` (sliced to one step of the fwd unit) whose unit-dim
    stride is adjusted so unit 1 lands on the mirrored step slot."""
    pairs = [list(p) for p in base.ap]
    pairs[1][0] += ustride_delta
    return AP(tensor=base.tensor, offset=base.offset + us * step, ap=pairs)


def _build_program():
    nc = bacc.Bacc("TRN2", target_bir_lowering=False, debug=False, num_devices=NCORES)
    emb_d = nc.dram_tensor("emb", [V, E], f32, kind="ExternalInput")
    words_d = nc.dram_tensor("words", [TB, 1], i32, kind="ExternalInput")
    aug_d = nc.dram_tensor("aug", [2, TB], bf16, kind="ExternalInput")
    mk_d = nc.dram_tensor("mk", [128, T, Bc], bf16, kind="ExternalInput")
    mk1_d = nc.dram_tensor("mk1", [128, T, Bc], bf16, kind="ExternalInput")
    wxd, wad, whd, xgd = {}, {}, {}, {}
    for u in UNITS:
        wxd[u] = nc.dram_tensor(f"w{u}x", [128, KCNT[u] * 1024], bf16, kind="ExternalInput")
        wad[u] = nc.dram_tensor(f"w{u}a", [2, 1024], bf16, kind="ExternalInput")
        whd[u] = nc.dram_tensor(f"w{u}h", [128, 2048], bf16, kind="ExternalInput")
        xgd[u] = nc.dram_tensor(f"xg{u}", [8, 128, TB], f32)
    clsx_d = nc.dram_tensor("clsx", [128, 4 * TAGS], bf16, kind="ExternalInput")
    clsb_d = nc.dram_tensor("clsb", [TAGS, 1], f32, kind="ExternalInput")
    logits_d = nc.dram_tensor("logits", [TAGS, TB], f32, kind="ExternalOutput")

    ctx = contextlib.ExitStack()
    with tile.TileContext(nc) as tc, ctx:
        pp = ctx.enter_context(tc.tile_pool(name="persist", bufs=1))
        xT = pp.tile([128, TB], bf16, tag="xT")
        aug_sb = pp.tile([2, TB], bf16, tag="aug")
        mk_sb = pp.tile([128, T, Bc], bf16, tag="mk")
        mk1_sb = pp.tile([128, T, Bc], bf16, tag="mk1")
        ident = pp.tile([128, 128], f32, tag="ident")
        wx_sb = {u: pp.tile([128, KCNT[u] * 1024], bf16, tag=f"wx{u}", name=f"wx{u}") for u in UNITS}
        wa_sb = {u: pp.tile([2, 1024], bf16, tag=f"wa{u}", name=f"wa{u}") for u in UNITS}
        wh_sb = {u: pp.tile([128, 2048], bf16, tag=f"wh{u}", name=f"wh{u}") for u in UNITS}
        cls_sb = pp.tile([128, 4 * TAGS], bf16, tag="clsx")
        clsb_sb = pp.tile([TAGS, 1], f32, tag="clsb")
        hs = {u: pp.tile([128, T, 2, Bc], bf16, tag=f"hs{u}", name=f"hs{u}") for u in UNITS}
        hcur = pp.tile([128, 2, 2, Bc], bf16, tag="hcur")   # [p, unit, hchunk, b]
        ccar = pp.tile([128, 2, 2, Bc], f32, tag="ccar")

        # ---- load weights / constants
        make_identity(nc, ident[:])
        for u in UNITS:
            nc.sync.dma_start(wx_sb[u][:], wxd[u][:])
            nc.sync.dma_start(wa_sb[u][:], wad[u][:])
            nc.sync.dma_start(wh_sb[u][:], whd[u][:])
        nc.sync.dma_start(cls_sb[:], clsx_d[:])
        nc.sync.dma_start(clsb_sb[:], clsb_d[:])
        nc.sync.dma_start(aug_sb[:], aug_d[:])
        nc.sync.dma_start(mk_sb[:], mk_d[:])
        nc.sync.dma_start(mk1_sb[:], mk1_d[:])

        # ---- embedding gather + transpose into xT
        with tc.tile_pool(name="gat", bufs=3) as gp, \
             tc.tile_pool(name="gps", bufs=3, space="PSUM") as gps:
            for n in range(TB // 128):
                idx = gp.tile([128, 1], i32, tag="idx")
                nc.sync.dma_start(idx[:], words_d[n * 128:(n + 1) * 128, :])
                xt = gp.tile([128, 128], f32, tag="xt")
                nc.gpsimd.indirect_dma_start(
                    out=xt[:], out_offset=None, in_=emb_d[:, :],
                    in_offset=bass.IndirectOffsetOnAxis(ap=idx[:, :1], axis=0))
                pst = gps.tile([128, 128], f32, tag="pst")
                nc.tensor.transpose(out=pst[:], in_=xt[:], identity=ident[:])
                nc.vector.tensor_copy(xT[:, n * 128:(n + 1) * 128], pst[:])

        # ---- xg precompute: xg[u][m] = Wx_m^T x + Wa_m^T aug, all t
        def xg_precompute(u, rhs_of_k):
            k_cnt = KCNT[u]
            with tc.tile_pool(name=f"xp{u}", bufs=4, space="PSUM") as xps, \
                 tc.tile_pool(name=f"xs{u}", bufs=4) as xsb:
                for n in range(TB // 512):
                    nsl = slice(n * 512, (n + 1) * 512)
                    for m in range(8):
                        psm = xps.tile([128, 512], f32, tag="ps")
                        for k in range(k_cnt):
                            nc.tensor.matmul(
                                out=psm[:],
                                lhsT=wx_sb[u][:, (k * 8 + m) * 128:(k * 8 + m + 1) * 128],
                                rhs=rhs_of_k(k, n),
                                start=(k == 0), stop=False)
                        nc.tensor.matmul(
                            out=psm[:],
                            lhsT=wa_sb[u][:, m * 128:(m + 1) * 128],
                            rhs=aug_sb[:, nsl],
                            start=False, stop=True)
                        stg = xsb.tile([128, 512], f32, tag="stg")
                        if (n + m) % 2 == 0:
                            nc.vector.tensor_copy(stg[:], psm[:])
                        else:
                            nc.scalar.activation(stg[:], psm[:],
                                                 mybir.ActivationFunctionType.Copy)
                        nc.sync.dma_start(xgd[u][m, :, nsl], stg[:])

        def l1_rhs(k, n):
            return xT[:, n * 512:(n + 1) * 512]

        xg_precompute("1f", l1_rhs)
        xg_precompute("1b", l1_rhs)

        # ---- fused recurrence for a (fwd, rev) unit pair
        def phase(units):
            uF, uR = units
            nc.vector.memset(hcur[:, :, :, :], 0.0)
            nc.vector.memset(ccar[:, :, :, :], 0.0)
            with tc.tile_pool(name=f"rc{uF}", bufs=2) as rp, \
                 tc.tile_pool(name=f"rps{uF}", bufs=4, space="PSUM") as rps, \
                 tc.tile_pool(name=f"rtmp{uF}", bufs=3) as tp:
                with tc.For_i(0, NBODY, hint_engines=(mybir.EngineType.PE,)) as i:
                    xb = rp.tile([128, 2, 8, 128], f32, tag="xb")
                    colF = i * 128
                    colR = i * (-128) + (TB - 128)
                    for m in range(8):
                        nc.sync.dma_start(xb[:, 0, m, :], xgd[uF][m, :, ds(colF, 128)])
                        nc.sync.dma_start(xb[:, 1, m, :], xgd[uR][m, :, ds(colR, 128)])
                    # hstage: [p, unit, slot, hchunk, b]; slot = local t index
                    hstage = rp.tile([128, 2, SPB, 2, Bc], bf16, tag="hst")
                    tF = i * SPB
                    tR = i * (-SPB) + (T - SPB)
                    for us in range(SPB):
                        psm = rps.tile([128, 2, 8, Bc], f32, tag="g")
                        for ui in range(2):
                            for m in range(8):
                                for k in range(2):
                                    nc.tensor.matmul(
                                        out=psm[:, ui, m, :],
                                        lhsT=wh_sb[units[ui]][:, (k * 8 + m) * 128:(k * 8 + m + 1) * 128],
                                        rhs=hcur[:, ui, k, :],
                                        start=(k == 0), stop=(k == 1))
                        g = tp.tile([128, 2, 8, Bc], f32, tag="gs")
                        # xb step slice: fwd unit cols us*8, rev unit cols (15-us)*8
                        in1 = _uap(xb[:, :, :, 0:Bc], us, (SPB - 1 - 2 * us) * Bc, Bc)
                        nc.vector.tensor_tensor(out=g[:, :, :, :], in0=psm[:, :, :, :],
                                                in1=in1, op=mybir.AluOpType.add)
                        sg = tp.tile([128, 2, 8, Bc], f32, tag="sg")
                        nc.scalar.activation(sg[:, :, 0:6, :], g[:, :, 0:6, :],
                                             mybir.ActivationFunctionType.Sigmoid)
                        nc.scalar.activation(sg[:, :, 6:8, :], g[:, :, 6:8, :],
                                             mybir.ActivationFunctionType.Tanh)
                        # csf first: only needs sigmoid -> overlaps the tanh
                        csf = tp.tile([128, 2, 2, Bc], f32, tag="csf")
                        nc.vector.tensor_tensor(out=csf[:, :, :, :], in0=sg[:, :, 2:4, :],
                                                in1=ccar[:, :, :, :], op=mybir.AluOpType.mult)
                        t1 = tp.tile([128, 2, 2, Bc], f32, tag="t1")
                        nc.vector.tensor_tensor(out=t1[:, :, :, :], in0=sg[:, :, 0:2, :],
                                                in1=sg[:, :, 6:8, :], op=mybir.AluOpType.mult)
                        nc.vector.tensor_tensor(out=ccar[:, :, :, :], in0=csf[:, :, :, :],
                                                in1=t1[:, :, :, :], op=mybir.AluOpType.add)
                        tc2 = tp.tile([128, 2, 2, Bc], f32, tag="tc2")
                        nc.scalar.activation(tc2[:, :, :, :], ccar[:, :, :, :],
                                             mybir.ActivationFunctionType.Tanh)
                        nc.vector.tensor_tensor(out=hcur[:, :, :, :], in0=sg[:, :, 4:6, :],
                                                in1=tc2[:, :, :, :], op=mybir.AluOpType.mult)
                        # stage h history off the critical chain (Pool engine);
                        # fwd unit -> slot us, rev unit -> slot 15-us
                        dst = _uap(hstage[:, :, 0, :, :], us,
                                   (SPB - 1 - 2 * us) * 2 * Bc, 2 * Bc)
                        nc.gpsimd.tensor_copy(dst, hcur[:, :, :, :])
                    nc.gpsimd.tensor_copy(hs[uF][:, ds(tF, SPB), :, :],
                                          hstage[:, 0, :, :, :])
                    nc.gpsimd.tensor_copy(hs[uR][:, ds(tR, SPB), :, :],
                                          hstage[:, 1, :, :, :])

        phase(("1f", "1b"))

        def l2_rhs(k, n):
            src = hs["1f"] if k < 2 else hs["1b"]
            return src[:, n * 64:(n + 1) * 64, k % 2, :]

        xg_precompute("2f", l2_rhs)
        xg_precompute("2b", l2_rhs)

        phase(("2f", "2b"))

        # ---- masked-h hold for the forward l2 output:
        # hold[t] = (1-m[t])*hold[t-1] + m[t]*h[t], done as scans in place.
        with tc.tile_pool(name="scan", bufs=1) as sp:
            hm = sp.tile([128, T, 2, Bc], bf16, tag="hm")
            for hc in range(2):
                nc.vector.tensor_tensor(out=hm[:, :, hc, :], in0=hs["2f"][:, :, hc, :],
                                        in1=mk_sb[:, :, :], op=mybir.AluOpType.mult)
            for hc in range(2):
                for b in range(Bc):
                    nc.vector.tensor_tensor_scan(
                        out=hs["2f"][:, :, hc, b],
                        data0=mk1_sb[:, :, b],
                        data1=hm[:, :, hc, b],
                        initial=0.0,
                        op0=mybir.AluOpType.mult,
                        op1=mybir.AluOpType.add)

            # ---- classifier
            with tc.tile_pool(name="cl", bufs=3) as cp, \
                 tc.tile_pool(name="cps", bufs=3, space="PSUM") as cps:
                for n in range(TB // 512):
                    psm = cps.tile([TAGS, 512], f32, tag="ps")
                    for k in range(4):
                        src = hs["2f"] if k < 2 else hs["2b"]
                        nc.tensor.matmul(
                            out=psm[:],
                            lhsT=cls_sb[:, k * TAGS:(k + 1) * TAGS],
                            rhs=src[:, n * 64:(n + 1) * 64, k % 2, :],
                            start=(k == 0), stop=(k == 3))
                    lg = cp.tile([TAGS, 512], f32, tag="lg")
                    nc.vector.tensor_scalar_add(lg[:], psm[:], clsb_sb[:, :1])
                    nc.sync.dma_start(logits_d[:, n * 512:(n + 1) * 512], lg[:])

    nc.compile()
    return nc


def kernel(**inputs):
    words = np.asarray(inputs["words"]).astype(np.int32)      # [B, T]
    lengths = np.asarray(inputs["lengths"]).astype(np.int32)  # [B]
    emb = np.asarray(inputs["emb"], dtype=np.float32)

    if "nc" not in _CACHE:
        _CACHE["nc"] = _build_program()
    nc = _CACHE["nc"]

    mask = (lengths[:, None] > np.arange(T)[None, :]).astype(np.float32)  # [B,T]
    wprep = {u: _prep_unit_weights(inputs[f"l{u}_Wih"], inputs[f"l{u}_Whh"],
                                   inputs[f"l{u}_bih"], inputs[f"l{u}_bhh"])
             for u in UNITS}
    clsW = np.asarray(inputs["cls_W"], dtype=np.float32)      # [50, 512]
    CT = clsW.T  # [512, 50]
    clsx = np.concatenate([CT[k * 128:(k + 1) * 128, :] for k in range(4)],
                          axis=1).astype(ml_dtypes.bfloat16)  # [128, 200]
    clsb = np.asarray(inputs["cls_b"], dtype=np.float32).reshape(TAGS, 1)

    in_maps = []
    for c in range(NCORES):
        bsl = slice(c * Bc, (c + 1) * Bc)
        w_c = words[bsl]                      # [Bc, T]
        m_c = mask[bsl]                       # [Bc, T]
        words_tm = np.ascontiguousarray(w_c.T).reshape(TB, 1)
        aug = np.stack([(1.0 - m_c.T).reshape(TB), np.ones(TB, np.float32)]
                       ).astype(ml_dtypes.bfloat16)           # [2, TB]
        mT = m_c.T.astype(ml_dtypes.bfloat16)                 # [T, Bc]
        mk = np.ascontiguousarray(np.broadcast_to(mT[None], (128, T, Bc)))
        mk1 = np.ascontiguousarray(
            np.broadcast_to((1.0 - m_c.T).astype(ml_dtypes.bfloat16)[None],
                            (128, T, Bc)))
        im = {"emb": emb, "words": words_tm, "aug": aug,
              "mk": mk, "mk1": mk1, "clsx": clsx, "clsb": clsb}
        for u in UNITS:
            wx, wa, wh = wprep[u]
            im[f"w{u}x"] = wx
            im[f"w{u}a"] = wa
            im[f"w{u}h"] = wh
        in_maps.append(im)

    _CACHE["in_maps"] = in_maps
    res = run_bass_kernel_spmd(nc, in_maps, list(range(NCORES)))
    out = np.empty((B, T, TAGS), np.float32)
    for c in range(NCORES):
        lg = res.results[c]["logits"]          # [50, TB], col = t*Bc + b
        out[c * Bc:(c + 1) * Bc] = lg.reshape(TAGS, T, Bc).transpose(2, 1, 0)
    return out


def bench(inputs):
    """Run once with NTFF tracing; returns HW exec_time_ns (and stashes trace)."""
    kernel(**inputs)  # ensure program built/cached
    nc = _CACHE["nc"]
    in_maps = _CACHE["in_maps"]
    import tempfile
    tmpdir = tempfile.mkdtemp(prefix="bilstm_trace_")
    res = run_bass_kernel_spmd(nc, in_maps, list(range(NCORES)), trace=True,
                               tmpdir=tmpdir)
    _CACHE["trace_dir"] = tmpdir
    _CACHE["last_bench"] = res
    print("trace dir:", tmpdir)
    return res.exec_time_ns


if __name__ == "__main__":
    import reference
    inputs = {k: np.asarray(v) for k, v in reference.setup_inputs().items()}
    got = kernel(**inputs)
    print(got.shape, got.dtype)


# revision 5
# speedup vs baseline: 1.8264x; 1.2095x over previous
"""BiLSTM tagger on 8 TRN2 NeuronCores.

Strategy (hardcoded for B=64,T=512,V=30000,E=128,H=256,TAGS=50):
  - Data-parallel: batch sharded 8 ways (8 sequences/core); weights replicated.
  - Per core: embedding gather (indirect DMA) -> PE transpose -> x^T in SBUF;
    input projections xg = W_ih_aug @ [x; 1-m; 1] precomputed for all t as big
    matmuls into DRAM scratch (bf16); recurrences as dynamic Tile loops with
    the two directions of a layer FUSED: one pointwise instruction covers both
    units ([128, 2u, chunk, Bc] APs), halving DVE/Act instruction count and
    keeping a single dependency chain.
  - Masking: (1-m) feature adds +/-60 to f/i gate pre-activations at masked
    steps, freezing c exactly. Backward h is then exactly 0 at masked steps.
    Forward l2 h at masked steps is repaired AFTER the recurrence with a
    tensor_tensor_scan per (hchunk, batch): hold[t] = (1-m)*hold[t-1]+m*h[t].
  - tanh(g) is computed as 2*sigmoid(2g)-1 by pre-doubling the g-gate rows:
    ONE sigmoid activation covers all 8 gate chunks. The cell state is
    carried as c/2 so the correction folds into existing multiplies
    (scalar_tensor_tensor) and tanh(c) = Tanh with scale=2.
  - Gate layout: gates on partitions (8 chunks of 128 = [i0 i1 f0 f1 o0 o1 g0
    g1]), batch on free dim; Whh stationary [128h x 128gate] bf16 tiles, h
    moving [128, 8]. Reverse-direction steps share the fused ops via custom
    APs whose unit-dim stride jumps to the mirrored column slot.
"""
import sys

sys.path.insert(0, "/opt/trn_rl_repo")
import contextlib

import numpy as np
import ml_dtypes

import concourse.bass as bass
import concourse.bacc as bacc
import concourse.mybir as mybir
import concourse.tile as tile
from concourse.ap import AP
from concourse.bass import ds
from concourse.bass_utils import run_bass_kernel_spmd
from concourse.masks import make_identity

B, T, V, E, H, TAGS = 64, 512, 30000, 128, 256, 50
NCORES = 8
Bc = B // NCORES          # 8 sequences per core
TB = T * Bc               # 4096 tokens per core
SPB = 32                  # steps per For_i body
NBODY = T // SPB          # 16
CPB = SPB * Bc            # xg columns per body (256)

f32 = mybir.dt.float32
bf16 = mybir.dt.bfloat16
i32 = mybir.dt.int32

UNITS = ("1f", "1b", "2f", "2b")
KCNT = {"1f": 1, "1b": 1, "2f": 4, "2b": 4}       # 128-row K chunks of x features
REV = {"1f": False, "1b": True, "2f": False, "2b": True}

# gate chunk order i0 i1 f0 f1 o0 o1 g0 g1 (torch row order is i f g o)
PERM = np.concatenate([np.arange(0, 256), np.arange(256, 512),
                       np.arange(768, 1024), np.arange(512, 768)])

_CACHE = {}


def _prep_unit_weights(Wih, Whh, bih, bhh):
    """Host-side weight marshalling for one LSTM direction (8 gate chunks).
    g-gate rows are doubled so tanh(g) = 2*sigmoid(2g)-1 on device."""
    din = Wih.shape[1]
    Wp = np.asarray(Wih)[PERM].astype(np.float64)   # [1024, din]
    Up = np.asarray(Whh)[PERM].astype(np.float64)   # [1024, 256]
    bp = (np.asarray(bih) + np.asarray(bhh)).astype(np.float64)[PERM]  # [1024]
    Wp[768:1024] *= 2.0
    Up[768:1024] *= 2.0
    bp[768:1024] *= 2.0
    M = 1024
    k_cnt = din // 128
    WT = Wp.T.astype(np.float32)        # [din, 1024]
    wx = np.concatenate([WT[k * 128:(k + 1) * 128, :] for k in range(k_cnt)],
                        axis=1).astype(ml_dtypes.bfloat16)  # [128, k_cnt*1024]
    # aug lhsT rows: feature0 = (1-m), feature1 = 1
    wa = np.zeros((2, M), np.float32)
    wa[0, 0:256] = -60.0   # i rows: -60*(1-m)
    wa[0, 256:512] = 60.0  # f rows: +60*(1-m)
    wa[1, :] = bp
    wa = wa.astype(ml_dtypes.bfloat16)
    UT = Up.T.astype(np.float32)
    wh = np.concatenate([UT[0:128, :], UT[128:256, :]], axis=1).astype(ml_dtypes.bfloat16)
    return wx, wa, wh


def _uap(base, us, ustride_delta, step):
    """AP over `base` (sliced to one step of the fwd unit) whose unit-dim
    stride is adjusted so unit 1 lands on the mirrored step slot."""
    pairs = [list(p) for p in base.ap]
    pairs[1][0] += ustride_delta
    return AP(tensor=base.tensor, offset=base.offset + us * step, ap=pairs)


def _build_program():
    nc = bacc.Bacc("TRN2", target_bir_lowering=False, debug=False, num_devices=NCORES)
    emb_d = nc.dram_tensor("emb", [V, E], f32, kind="ExternalInput")
    words_d = nc.dram_tensor("words", [128, TB // 128], i32, kind="ExternalInput")
    aug_d = nc.dram_tensor("aug", [2, TB], bf16, kind="ExternalInput")
    mk_d = nc.dram_tensor("mk", [128, T, Bc], bf16, kind="ExternalInput")
    mk1_d = nc.dram_tensor("mk1", [128, T, Bc], bf16, kind="ExternalInput")
    wxd, wad, whd, xgd = {}, {}, {}, {}
    for u in UNITS:
        wxd[u] = nc.dram_tensor(f"w{u}x", [128, KCNT[u] * 1024], bf16, kind="ExternalInput")
        wad[u] = nc.dram_tensor(f"w{u}a", [2, 1024], bf16, kind="ExternalInput")
        whd[u] = nc.dram_tensor(f"w{u}h", [128, 2048], bf16, kind="ExternalInput")
        xgd[u] = nc.dram_tensor(f"xg{u}", [8, 128, TB], bf16)
    clsx_d = nc.dram_tensor("clsx", [128, 4 * TAGS], bf16, kind="ExternalInput")
    clsb_d = nc.dram_tensor("clsb", [TAGS, 1], f32, kind="ExternalInput")
    logits_d = nc.dram_tensor("logits", [TAGS, TB], f32, kind="ExternalOutput")

    ctx = contextlib.ExitStack()
    with tile.TileContext(nc) as tc, ctx:
        pp = ctx.enter_context(tc.tile_pool(name="persist", bufs=1))
        xT = pp.tile([128, TB], bf16, tag="xT")
        aug_sb = pp.tile([2, TB], bf16, tag="aug")
        mk_sb = pp.tile([128, T, Bc], bf16, tag="mk")
        mk1_sb = pp.tile([128, T, Bc], bf16, tag="mk1")
        ident = pp.tile([128, 128], f32, tag="ident")
        wx_sb = {u: pp.tile([128, KCNT[u] * 1024], bf16, tag=f"wx{u}", name=f"wx{u}") for u in UNITS}
        wa_sb = {u: pp.tile([2, 1024], bf16, tag=f"wa{u}", name=f"wa{u}") for u in UNITS}
        wh_sb = {u: pp.tile([128, 2048], bf16, tag=f"wh{u}", name=f"wh{u}") for u in UNITS}
        cls_sb = pp.tile([128, 4 * TAGS], bf16, tag="clsx")
        clsb_sb = pp.tile([TAGS, 1], f32, tag="clsb")
        hs = {u: pp.tile([128, T, 2, Bc], bf16, tag=f"hs{u}", name=f"hs{u}") for u in UNITS}
        hcur = pp.tile([128, 2, 2, Bc], bf16, tag="hcur")   # [p, unit, hchunk, b]
        ccar = pp.tile([128, 2, 2, Bc], f32, tag="ccar")    # carries c/2

        # ---- load weights / constants
        make_identity(nc, ident[:])
        for u in UNITS:
            nc.sync.dma_start(wx_sb[u][:], wxd[u][:])
            nc.sync.dma_start(wa_sb[u][:], wad[u][:])
            nc.sync.dma_start(wh_sb[u][:], whd[u][:])
        nc.sync.dma_start(cls_sb[:], clsx_d[:])
        nc.sync.dma_start(clsb_sb[:], clsb_d[:])
        nc.sync.dma_start(aug_sb[:], aug_d[:])
        nc.sync.dma_start(mk_sb[:], mk_d[:])
        nc.sync.dma_start(mk1_sb[:], mk1_d[:])

        # ---- embedding gather + transpose into xT
        with tc.tile_pool(name="gat", bufs=3) as gp, \
             tc.tile_pool(name="gps", bufs=3, space="PSUM") as gps:
            idx_all = gp.tile([128, TB // 128], i32, tag="idxall")
            nc.sync.dma_start(idx_all[:], words_d[:])
            for n in range(TB // 128):
                xt = gp.tile([128, 128], f32, tag="xt")
                nc.gpsimd.indirect_dma_start(
                    out=xt[:], out_offset=None, in_=emb_d[:, :],
                    in_offset=bass.IndirectOffsetOnAxis(ap=idx_all[:, n:n + 1], axis=0))
                pst = gps.tile([128, 128], f32, tag="pst")
                nc.tensor.transpose(out=pst[:], in_=xt[:], identity=ident[:])
                nc.vector.tensor_copy(xT[:, n * 128:(n + 1) * 128], pst[:])

        # ---- xg precompute: xg[u][m] = Wx_m^T x + Wa_m^T aug, all t
        # 4 psum banks per group of 4 m-chunks; one staging copy + one DMA per group
        def xg_precompute(u, rhs_of_k):
            k_cnt = KCNT[u]
            with tc.tile_pool(name=f"xp{u}", bufs=2, space="PSUM") as xps, \
                 tc.tile_pool(name=f"xs{u}", bufs=2) as xsb:
                for n in range(TB // 512):
                    nsl = slice(n * 512, (n + 1) * 512)
                    for mg in range(2):
                        psm = xps.tile([128, 4, 512], f32, tag="ps")
                        for mi in range(4):
                            m = mg * 4 + mi
                            for k in range(k_cnt):
                                nc.tensor.matmul(
                                    out=psm[:, mi, :],
                                    lhsT=wx_sb[u][:, (k * 8 + m) * 128:(k * 8 + m + 1) * 128],
                                    rhs=rhs_of_k(k, n),
                                    start=(k == 0), stop=False)
                            nc.tensor.matmul(
                                out=psm[:, mi, :],
                                lhsT=wa_sb[u][:, m * 128:(m + 1) * 128],
                                rhs=aug_sb[:, nsl],
                                start=False, stop=True)
                        stg = xsb.tile([128, 4, 512], bf16, tag="stg")
                        if mg == 0:
                            nc.vector.tensor_copy(stg[:], psm[:])
                        else:
                            nc.scalar.activation(stg[:], psm[:],
                                                 mybir.ActivationFunctionType.Copy)
                        nc.sync.dma_start(
                            xgd[u][mg * 4:(mg + 1) * 4, :, nsl].transpose([1, 0, 2]),
                            stg[:])

        def l1_rhs(k, n):
            return xT[:, n * 512:(n + 1) * 512]

        xg_precompute("1f", l1_rhs)
        xg_precompute("1b", l1_rhs)

        # ---- fused recurrence for a (fwd, rev) unit pair
        def phase(units):
            uF, uR = units
            nc.vector.memset(hcur[:, :, :, :], 0.0)
            nc.vector.memset(ccar[:, :, :, :], 0.0)
            with tc.tile_pool(name=f"rc{uF}", bufs=2) as rp, \
                 tc.tile_pool(name=f"rps{uF}", bufs=4, space="PSUM") as rps, \
                 tc.tile_pool(name=f"rtmp{uF}", bufs=3) as tp:
                with tc.For_i(0, NBODY, hint_engines=(mybir.EngineType.PE,)) as i:
                    xb = rp.tile([128, 2, 8, CPB], bf16, tag="xb")
                    colF = i * CPB
                    colR = i * (-CPB) + (TB - CPB)
                    nc.sync.dma_start(xb[:, 0, :, :],
                                      xgd[uF][:, :, ds(colF, CPB)].transpose([1, 0, 2]))
                    nc.sync.dma_start(xb[:, 1, :, :],
                                      xgd[uR][:, :, ds(colR, CPB)].transpose([1, 0, 2]))
                    # hstage: [p, unit, slot, hchunk, b]; slot = local t index
                    hstage = rp.tile([128, 2, SPB, 2, Bc], bf16, tag="hst")
                    tF = i * SPB
                    tR = i * (-SPB) + (T - SPB)
                    for us in range(SPB):
                        psm = rps.tile([128, 2, 8, Bc], f32, tag="g")
                        for ui in range(2):
                            for m in range(8):
                                for k in range(2):
                                    nc.tensor.matmul(
                                        out=psm[:, ui, m, :],
                                        lhsT=wh_sb[units[ui]][:, (k * 8 + m) * 128:(k * 8 + m + 1) * 128],
                                        rhs=hcur[:, ui, k, :],
                                        start=(k == 0), stop=(k == 1))
                        g = tp.tile([128, 2, 8, Bc], f32, tag="gs")
                        # xb step slice: fwd unit cols us*8, rev unit cols (SPB-1-us)*8
                        in1 = _uap(xb[:, :, :, 0:Bc], us, (SPB - 1 - 2 * us) * Bc, Bc)
                        nc.vector.tensor_tensor(out=g[:, :, :, :], in0=psm[:, :, :, :],
                                                in1=in1, op=mybir.AluOpType.add)
                        # one sigmoid for all 8 chunks (g rows pre-doubled)
                        sg = tp.tile([128, 2, 8, Bc], f32, tag="sg")
                        nc.scalar.activation(sg[:, :, :, :], g[:, :, :, :],
                                             mybir.ActivationFunctionType.Sigmoid)
                        csf = tp.tile([128, 2, 2, Bc], f32, tag="csf")
                        nc.vector.tensor_tensor(out=csf[:, :, :, :], in0=sg[:, :, 2:4, :],
                                                in1=ccar[:, :, :, :], op=mybir.AluOpType.mult)
                        # t1h = (sigmoid(2g)-0.5) * sigmoid(i)  ==  i_act*tanh(g)/2
                        t1 = tp.tile([128, 2, 2, Bc], f32, tag="t1")
                        nc.vector.scalar_tensor_tensor(
                            out=t1[:, :, :, :], in0=sg[:, :, 6:8, :], scalar=0.5,
                            in1=sg[:, :, 0:2, :],
                            op0=mybir.AluOpType.subtract, op1=mybir.AluOpType.mult)
                        nc.vector.tensor_tensor(out=ccar[:, :, :, :], in0=csf[:, :, :, :],
                                                in1=t1[:, :, :, :], op=mybir.AluOpType.add)
                        # tanh(c) = Tanh(2 * (c/2))
                        tc2 = tp.tile([128, 2, 2, Bc], f32, tag="tc2")
                        nc.scalar.activation(tc2[:, :, :, :], ccar[:, :, :, :],
                                             mybir.ActivationFunctionType.Tanh,
                                             scale=2.0)
                        nc.vector.tensor_tensor(out=hcur[:, :, :, :], in0=sg[:, :, 4:6, :],
                                                in1=tc2[:, :, :, :], op=mybir.AluOpType.mult)
                        # stage h history off the critical chain (Pool engine);
                        # fwd unit -> slot us, rev unit -> slot SPB-1-us
                        dst = _uap(hstage[:, :, 0, :, :], us,
                                   (SPB - 1 - 2 * us) * 2 * Bc, 2 * Bc)
                        nc.gpsimd.tensor_copy(dst, hcur[:, :, :, :])
                    nc.gpsimd.tensor_copy(hs[uF][:, ds(tF, SPB), :, :],
                                          hstage[:, 0, :, :, :])
                    nc.gpsimd.tensor_copy(hs[uR][:, ds(tR, SPB), :, :],
                                          hstage[:, 1, :, :, :])

        phase(("1f", "1b"))

        def l2_rhs(k, n):
            src = hs["1f"] if k < 2 else hs["1b"]
            return src[:, n * 64:(n + 1) * 64, k % 2, :]

        xg_precompute("2f", l2_rhs)
        xg_precompute("2b", l2_rhs)

        phase(("2f", "2b"))

        # ---- masked-h hold for the forward l2 output:
        # hold[t] = (1-m[t])*hold[t-1] + m[t]*h[t], done as scans in place.
        with tc.tile_pool(name="scan", bufs=1) as sp:
            hm = sp.tile([128, T, 2, Bc], bf16, tag="hm")
            for hc in range(2):
                nc.vector.tensor_tensor(out=hm[:, :, hc, :], in0=hs["2f"][:, :, hc, :],
                                        in1=mk_sb[:, :, :], op=mybir.AluOpType.mult)
            for hc in range(2):
                for b in range(Bc):
                    nc.vector.tensor_tensor_scan(
                        out=hs["2f"][:, :, hc, b],
                        data0=mk1_sb[:, :, b],
                        data1=hm[:, :, hc, b],
                        initial=0.0,
                        op0=mybir.AluOpType.mult,
                        op1=mybir.AluOpType.add)

            # ---- classifier
            with tc.tile_pool(name="cl", bufs=3) as cp, \
                 tc.tile_pool(name="cps", bufs=3, space="PSUM") as cps:
                for n in range(TB // 512):
                    psm = cps.tile([TAGS, 512], f32, tag="ps")
                    for k in range(4):
                        src = hs["2f"] if k < 2 else hs["2b"]
                        nc.tensor.matmul(
                            out=psm[:],
                            lhsT=cls_sb[:, k * TAGS:(k + 1) * TAGS],
                            rhs=src[:, n * 64:(n + 1) * 64, k % 2, :],
                            start=(k == 0), stop=(k == 3))
                    lg = cp.tile([TAGS, 512], f32, tag="lg")
                    nc.vector.tensor_scalar_add(lg[:], psm[:], clsb_sb[:, :1])
                    nc.sync.dma_start(logits_d[:, n * 512:(n + 1) * 512], lg[:])

    nc.compile()
    return nc


def kernel(**inputs):
    words = np.asarray(inputs["words"]).astype(np.int32)      # [B, T]
    lengths = np.asarray(inputs["lengths"]).astype(np.int32)  # [B]
    emb = np.asarray(inputs["emb"], dtype=np.float32)

    if "nc" not in _CACHE:
        _CACHE["nc"] = _build_program()
    nc = _CACHE["nc"]

    mask = (lengths[:, None] > np.arange(T)[None, :]).astype(np.float32)  # [B,T]
    wprep = {u: _prep_unit_weights(inputs[f"l{u}_Wih"], inputs[f"l{u}_Whh"],
                                   inputs[f"l{u}_bih"], inputs[f"l{u}_bhh"])
             for u in UNITS}
    clsW = np.asarray(inputs["cls_W"], dtype=np.float32)      # [50, 512]
    CT = clsW.T  # [512, 50]
    clsx = np.concatenate([CT[k * 128:(k + 1) * 128, :] for k in range(4)],
                          axis=1).astype(ml_dtypes.bfloat16)  # [128, 200]
    clsb = np.asarray(inputs["cls_b"], dtype=np.float32).reshape(TAGS, 1)

    in_maps = []
    for c in range(NCORES):
        bsl = slice(c * Bc, (c + 1) * Bc)
        w_c = words[bsl]                      # [Bc, T]
        m_c = mask[bsl]                       # [Bc, T]
        words_tm = np.ascontiguousarray(
            np.ascontiguousarray(w_c.T).reshape(TB // 128, 128).T)  # [128, 32]
        aug = np.stack([(1.0 - m_c.T).reshape(TB), np.ones(TB, np.float32)]
                       ).astype(ml_dtypes.bfloat16)           # [2, TB]
        mT = m_c.T.astype(ml_dtypes.bfloat16)                 # [T, Bc]
        mk = np.ascontiguousarray(np.broadcast_to(mT[None], (128, T, Bc)))
        mk1 = np.ascontiguousarray(
            np.broadcast_to((1.0 - m_c.T).astype(ml_dtypes.bfloat16)[None],
                            (128, T, Bc)))
        im = {"emb": emb, "words": words_tm, "aug": aug,
              "mk": mk, "mk1": mk1, "clsx": clsx, "clsb": clsb}
        for u in UNITS:
            wx, wa, wh = wprep[u]
            im[f"w{u}x"] = wx
            im[f"w{u}a"] = wa
            im[f"w{u}h"] = wh
        in_maps.append(im)

    _CACHE["in_maps"] = in_maps
    res = run_bass_kernel_spmd(nc, in_maps, list(range(NCORES)))
    out = np.empty((B, T, TAGS), np.float32)
    for c in range(NCORES):
        lg = res.results[c]["logits"]          # [50, TB], col = t*Bc + b
        out[c * Bc:(c + 1) * Bc] = lg.reshape(TAGS, T, Bc).transpose(2, 1, 0)
    return out


def bench(inputs):
    """Run once with NTFF tracing; returns HW exec_time_ns (and stashes trace)."""
    kernel(**inputs)  # ensure program built/cached
    nc = _CACHE["nc"]
    in_maps = _CACHE["in_maps"]
    import tempfile
    tmpdir = tempfile.mkdtemp(prefix="bilstm_trace_")
    res = run_bass_kernel_spmd(nc, in_maps, list(range(NCORES)), trace=True,
                               tmpdir=tmpdir)
    _CACHE["trace_dir"] = tmpdir
    _CACHE["last_bench"] = res
    print("trace dir:", tmpdir)
    return res.exec_time_ns


if __name__ == "__main__":
    import reference
    inputs = {k: np.asarray(v) for k, v in reference.setup_inputs().items()}
    got = kernel(**inputs)
    print(got.shape, got.dtype)


# revision 11
# speedup vs baseline: 1.9869x; 1.0879x over previous
"""BiLSTM tagger on 8 TRN2 NeuronCores.

Strategy (hardcoded for B=64,T=512,V=30000,E=128,H=256,TAGS=50):
  - Data-parallel: batch sharded 8 ways (8 sequences/core); weights replicated.
  - Per core: embedding gather (indirect DMA) -> PE transpose -> x^T in SBUF;
    input projections xg = W_ih_aug @ [x; 1-m; 1] precomputed for all t as big
    matmuls into DRAM scratch (bf16); recurrences as dynamic Tile loops with
    the two directions of a layer FUSED: one pointwise instruction covers both
    units ([128, 2u, chunk, Bc] APs), halving DVE/Act instruction count and
    keeping a single dependency chain.
  - Masking: (1-m) feature adds +/-60 to f/i gate pre-activations at masked
    steps, freezing c exactly. Backward h is then exactly 0 at masked steps.
    Forward l2 h at masked steps is repaired AFTER the recurrence with a
    tensor_tensor_scan per (hchunk, batch): hold[t] = (1-m)*hold[t-1]+m*h[t].
  - tanh(g) is computed as 2*sigmoid(2g)-1 by pre-doubling the g-gate rows:
    ONE sigmoid activation covers all 8 gate chunks. The cell state is
    carried as c/2 so the correction folds into existing multiplies
    (scalar_tensor_tensor) and tanh(c) = Tanh with scale=2.
  - Gate layout: gates on partitions (8 chunks of 128 = [i0 i1 f0 f1 o0 o1 g0
    g1]), batch on free dim; Whh stationary [128h x 128gate] bf16 tiles, h
    moving [128, 8]. Reverse-direction steps share the fused ops via custom
    APs whose unit-dim stride jumps to the mirrored column slot.
"""
import sys

sys.path.insert(0, "/opt/trn_rl_repo")
import contextlib

import numpy as np
import ml_dtypes

import concourse.bass as bass
import concourse.bacc as bacc
import concourse.mybir as mybir
import concourse.tile as tile
from concourse.ap import AP
from concourse.bass import ds
from concourse.bass_utils import run_bass_kernel_spmd
from concourse.masks import make_identity

B, T, V, E, H, TAGS = 64, 512, 30000, 128, 256, 50
NCORES = 8
Bc = B // NCORES          # 8 sequences per core
TB = T * Bc               # 4096 tokens per core
SPB = 32                  # steps per For_i body
NBODY = T // SPB          # 16
CPB = SPB * Bc            # xg columns per body (256)

f32 = mybir.dt.float32
bf16 = mybir.dt.bfloat16
i32 = mybir.dt.int32

UNITS = ("1f", "1b", "2f", "2b")
KCNT = {"1f": 1, "1b": 1, "2f": 4, "2b": 4}       # 128-row K chunks of x features
REV = {"1f": False, "1b": True, "2f": False, "2b": True}

# gate chunk order i0 i1 f0 f1 o0 o1 g0 g1 (torch row order is i f g o)
PERM = np.concatenate([np.arange(0, 256), np.arange(256, 512),
                       np.arange(768, 1024), np.arange(512, 768)])

_CACHE = {}


def _prep_unit_weights(Wih, Whh, bih, bhh):
    """Host-side weight marshalling for one LSTM direction (8 gate chunks).
    g-gate rows are doubled so tanh(g) = 2*sigmoid(2g)-1 on device."""
    din = Wih.shape[1]
    Wp = np.asarray(Wih)[PERM].astype(np.float64)   # [1024, din]
    Up = np.asarray(Whh)[PERM].astype(np.float64)   # [1024, 256]
    bp = (np.asarray(bih) + np.asarray(bhh)).astype(np.float64)[PERM]  # [1024]
    Wp[768:1024] *= 2.0
    Up[768:1024] *= 2.0
    bp[768:1024] *= 2.0
    M = 1024
    k_cnt = din // 128
    WT = Wp.T.astype(np.float32)        # [din, 1024]
    wx = np.concatenate([WT[k * 128:(k + 1) * 128, :] for k in range(k_cnt)],
                        axis=1).astype(ml_dtypes.bfloat16)  # [128, k_cnt*1024]
    # aug lhsT rows: feature0 = (1-m), feature1 = 1
    wa = np.zeros((2, M), np.float32)
    wa[0, 0:256] = -60.0   # i rows: -60*(1-m)
    wa[0, 256:512] = 60.0  # f rows: +60*(1-m)
    wa[1, :] = bp
    wa = wa.astype(ml_dtypes.bfloat16)
    UT = Up.T.astype(np.float32)
    wh = np.concatenate([UT[0:128, :], UT[128:256, :]], axis=1).astype(ml_dtypes.bfloat16)
    return wx, wa, wh


def _uap(base, us, ustride_delta, step):
    """AP over `base` (sliced to one step of the fwd unit) whose unit-dim
    stride is adjusted so unit 1 lands on the mirrored step slot."""
    pairs = [list(p) for p in base.ap]
    pairs[1][0] += ustride_delta
    return AP(tensor=base.tensor, offset=base.offset + us * step, ap=pairs)


def _build_program():
    nc = bacc.Bacc("TRN2", target_bir_lowering=False, debug=False, num_devices=NCORES)
    emb_d = nc.dram_tensor("emb", [V, E], f32, kind="ExternalInput")
    words_d = nc.dram_tensor("words", [128, TB // 128], i32, kind="ExternalInput")
    aug_d = nc.dram_tensor("aug", [2, TB], bf16, kind="ExternalInput")
    mk_d = nc.dram_tensor("mk", [128, T, Bc], bf16, kind="ExternalInput")
    mk1_d = nc.dram_tensor("mk1", [128, T, Bc], bf16, kind="ExternalInput")
    wxd, wad, whd, xgd = {}, {}, {}, {}
    for u in UNITS:
        wxd[u] = nc.dram_tensor(f"w{u}x", [128, KCNT[u] * 1024], bf16, kind="ExternalInput")
        wad[u] = nc.dram_tensor(f"w{u}a", [2, 1024], bf16, kind="ExternalInput")
        whd[u] = nc.dram_tensor(f"w{u}h", [128, 2048], bf16, kind="ExternalInput")
        xgd[u] = nc.dram_tensor(f"xg{u}", [8, 128, TB], bf16)
    clsx_d = nc.dram_tensor("clsx", [128, 4 * TAGS], bf16, kind="ExternalInput")
    clsb_d = nc.dram_tensor("clsb", [TAGS, 1], f32, kind="ExternalInput")
    logits_d = nc.dram_tensor("logits", [TAGS, TB], f32, kind="ExternalOutput")

    ctx = contextlib.ExitStack()
    with tile.TileContext(nc) as tc, ctx:
        pp = ctx.enter_context(tc.tile_pool(name="persist", bufs=1))
        xT = pp.tile([128, TB], bf16, tag="xT")
        aug_sb = pp.tile([2, TB], bf16, tag="aug")
        mk_sb = pp.tile([128, T, Bc], bf16, tag="mk")
        mk1_sb = pp.tile([128, T, Bc], bf16, tag="mk1")
        ident = pp.tile([128, 128], f32, tag="ident")
        wx_sb = {u: pp.tile([128, KCNT[u] * 1024], bf16, tag=f"wx{u}", name=f"wx{u}") for u in UNITS}
        wa_sb = {u: pp.tile([2, 1024], bf16, tag=f"wa{u}", name=f"wa{u}") for u in UNITS}
        wh_sb = {u: pp.tile([128, 2048], bf16, tag=f"wh{u}", name=f"wh{u}") for u in UNITS}
        cls_sb = pp.tile([128, 4 * TAGS], bf16, tag="clsx")
        clsb_sb = pp.tile([TAGS, 1], f32, tag="clsb")
        hs = {u: pp.tile([128, T, 2, Bc], bf16, tag=f"hs{u}", name=f"hs{u}") for u in UNITS}
        hcur = pp.tile([128, 2, 2, Bc], bf16, tag="hcur")   # [p, unit, hchunk, b]
        ccar = pp.tile([128, 2, 2, Bc], f32, tag="ccar")    # carries c/2
        identb = pp.tile([128, 128], bf16, tag="identb")

        # ---- load weights / constants
        make_identity(nc, ident[:])
        make_identity(nc, identb[:])
        for u in UNITS:
            nc.sync.dma_start(wx_sb[u][:], wxd[u][:])
            nc.sync.dma_start(wa_sb[u][:], wad[u][:])
            nc.sync.dma_start(wh_sb[u][:], whd[u][:])
        nc.sync.dma_start(cls_sb[:], clsx_d[:])
        nc.sync.dma_start(clsb_sb[:], clsb_d[:])
        nc.sync.dma_start(aug_sb[:], aug_d[:])
        nc.sync.dma_start(mk_sb[:], mk_d[:])
        nc.sync.dma_start(mk1_sb[:], mk1_d[:])

        # ---- embedding gather + transpose into xT
        with tc.tile_pool(name="gat", bufs=3) as gp, \
             tc.tile_pool(name="gps", bufs=3, space="PSUM") as gps:
            idx_all = gp.tile([128, TB // 128], i32, tag="idxall")
            nc.sync.dma_start(idx_all[:], words_d[:])
            for n in range(TB // 128):
                xt = gp.tile([128, 128], f32, tag="xt")
                nc.gpsimd.indirect_dma_start(
                    out=xt[:], out_offset=None, in_=emb_d[:, :],
                    in_offset=bass.IndirectOffsetOnAxis(ap=idx_all[:, n:n + 1], axis=0))
                pst = gps.tile([128, 128], f32, tag="pst")
                nc.tensor.transpose(out=pst[:], in_=xt[:], identity=ident[:])
                nc.vector.tensor_copy(xT[:, n * 128:(n + 1) * 128], pst[:])

        # ---- xg precompute: xg[u][m] = Wx_m^T x + Wa_m^T aug, all t
        # 4 psum banks per group of 4 m-chunks; one staging copy + one DMA per group
        def xg_precompute(u, rhs_of_k):
            k_cnt = KCNT[u]
            with tc.tile_pool(name=f"xp{u}", bufs=2, space="PSUM") as xps, \
                 tc.tile_pool(name=f"xs{u}", bufs=2) as xsb:
                for n in range(TB // 512):
                    nsl = slice(n * 512, (n + 1) * 512)
                    for mg in range(2):
                        psm = xps.tile([128, 4, 512], f32, tag="ps")
                        for mi in range(4):
                            m = mg * 4 + mi
                            for k in range(k_cnt):
                                nc.tensor.matmul(
                                    out=psm[:, mi, :],
                                    lhsT=wx_sb[u][:, (k * 8 + m) * 128:(k * 8 + m + 1) * 128],
                                    rhs=rhs_of_k(k, n),
                                    start=(k == 0), stop=False)
                            nc.tensor.matmul(
                                out=psm[:, mi, :],
                                lhsT=wa_sb[u][:, m * 128:(m + 1) * 128],
                                rhs=aug_sb[:, nsl],
                                start=False, stop=True)
                        stg = xsb.tile([128, 4, 512], bf16, tag="stg")
                        if mg == 0:
                            nc.vector.tensor_copy(stg[:], psm[:])
                        else:
                            nc.scalar.activation(stg[:], psm[:],
                                                 mybir.ActivationFunctionType.Copy)
                        nc.sync.dma_start(
                            xgd[u][mg * 4:(mg + 1) * 4, :, nsl].transpose([1, 0, 2]),
                            stg[:])

        def l1_rhs(k, n):
            return xT[:, n * 512:(n + 1) * 512]

        xg_precompute("1f", l1_rhs)
        xg_precompute("1b", l1_rhs)

        # ---- fused recurrence for a (fwd, rev) unit pair
        def phase(units):
            uF, uR = units
            nc.vector.memset(hcur[:, :, :, :], 0.0)
            nc.vector.memset(ccar[:, :, :, :], 0.0)
            with tc.tile_pool(name=f"rc{uF}", bufs=2) as rp, \
                 tc.tile_pool(name=f"rps{uF}", bufs=4, space="PSUM") as rps, \
                 tc.tile_pool(name=f"rtmp{uF}", bufs=3) as tp:
                with tc.For_i(0, NBODY, hint_engines=(mybir.EngineType.PE,)) as i:
                    xb = rp.tile([128, 2, 8, CPB], bf16, tag="xb")
                    colF = i * CPB
                    colR = i * (-CPB) + (TB - CPB)
                    nc.sync.dma_start(xb[:, 0, :, :],
                                      xgd[uF][:, :, ds(colF, CPB)].transpose([1, 0, 2]))
                    nc.sync.dma_start(xb[:, 1, :, :],
                                      xgd[uR][:, :, ds(colR, CPB)].transpose([1, 0, 2]))
                    # hstage: [p, unit, slot, hchunk, b]; slot = local t index
                    hstage = rp.tile([128, 2, SPB, 2, Bc], bf16, tag="hst")
                    tF = i * SPB
                    tR = i * (-SPB) + (T - SPB)
                    for us in range(SPB):
                        # padded to a full 2KB PSUM bank so each rotating buffer
                        # occupies its own bank: the next step's xb matmuls write
                        # concurrently with this step's activation read
                        psm = rps.tile([128, 2, 8, 32], f32, tag="g")
                        # xb contribution as ONE identity matmul (the whole step's
                        # [2u, 8m, 8b] slice via a mirrored-unit-stride AP): it only
                        # depends on the xb DMA, so the PE runs it during the
                        # previous step's pointwise. The h matmuls then accumulate
                        # into the same single open group (one start per bank —
                        # interleaved open groups corrupt PSUM).
                        in1 = _uap(xb[:, :, :, 0:Bc], us, (SPB - 1 - 2 * us) * Bc, Bc)
                        nc.tensor.matmul(
                            out=psm[:, :, :, 0:Bc],
                            lhsT=identb[:, :],
                            rhs=in1,
                            start=True, stop=False, skip_group_check=True)
                        for ui in range(2):
                            for m in range(8):
                                for k in range(2):
                                    nc.tensor.matmul(
                                        out=psm[:, ui, m, 0:Bc],
                                        lhsT=wh_sb[units[ui]][:, (k * 8 + m) * 128:(k * 8 + m + 1) * 128],
                                        rhs=hcur[:, ui, k, :],
                                        start=False,
                                        stop=(ui == 1 and m == 7 and k == 1),
                                        skip_group_check=True)
                        # one sigmoid for all 8 chunks (g rows pre-doubled),
                        # reading gate pre-activations straight from PSUM
                        sg = tp.tile([128, 2, 8, Bc], f32, tag="sg")
                        nc.scalar.activation(sg[:, :, :, :], psm[:, :, :, 0:Bc],
                                             mybir.ActivationFunctionType.Sigmoid)
                        csf = tp.tile([128, 2, 2, Bc], f32, tag="csf")
                        nc.vector.tensor_tensor(out=csf[:, :, :, :], in0=sg[:, :, 2:4, :],
                                                in1=ccar[:, :, :, :], op=mybir.AluOpType.mult)
                        # t1h = (sigmoid(2g)-0.5) * sigmoid(i)  ==  i_act*tanh(g)/2
                        t1 = tp.tile([128, 2, 2, Bc], f32, tag="t1")
                        nc.vector.scalar_tensor_tensor(
                            out=t1[:, :, :, :], in0=sg[:, :, 6:8, :], scalar=0.5,
                            in1=sg[:, :, 0:2, :],
                            op0=mybir.AluOpType.subtract, op1=mybir.AluOpType.mult)
                        nc.vector.tensor_tensor(out=ccar[:, :, :, :], in0=csf[:, :, :, :],
                                                in1=t1[:, :, :, :], op=mybir.AluOpType.add)
                        # tanh(c) = Tanh(2 * (c/2))
                        tc2 = tp.tile([128, 2, 2, Bc], f32, tag="tc2")
                        nc.scalar.activation(tc2[:, :, :, :], ccar[:, :, :, :],
                                             mybir.ActivationFunctionType.Tanh,
                                             scale=2.0)
                        nc.vector.tensor_tensor(out=hcur[:, :, :, :], in0=sg[:, :, 4:6, :],
                                                in1=tc2[:, :, :, :], op=mybir.AluOpType.mult)
                        # stage h history off the critical chain (Pool engine);
                        # fwd unit -> slot us, rev unit -> slot SPB-1-us
                        dst = _uap(hstage[:, :, 0, :, :], us,
                                   (SPB - 1 - 2 * us) * 2 * Bc, 2 * Bc)
                        nc.gpsimd.tensor_copy(dst, hcur[:, :, :, :])
                    nc.scalar.activation(hs[uF][:, ds(tF, SPB), :, :],
                                         hstage[:, 0, :, :, :],
                                         mybir.ActivationFunctionType.Copy)
                    nc.scalar.activation(hs[uR][:, ds(tR, SPB), :, :],
                                         hstage[:, 1, :, :, :],
                                         mybir.ActivationFunctionType.Copy)

        phase(("1f", "1b"))

        def l2_rhs(k, n):
            src = hs["1f"] if k < 2 else hs["1b"]
            return src[:, n * 64:(n + 1) * 64, k % 2, :]

        xg_precompute("2f", l2_rhs)
        xg_precompute("2b", l2_rhs)

        phase(("2f", "2b"))

        # ---- masked-h hold for the forward l2 output:
        # hold[t] = (1-m[t])*hold[t-1] + m[t]*h[t], done as scans in place.
        with tc.tile_pool(name="scan", bufs=1) as sp:
            hm = sp.tile([128, T, 2, Bc], bf16, tag="hm")
            for hc in range(2):
                nc.vector.tensor_tensor(out=hm[:, :, hc, :], in0=hs["2f"][:, :, hc, :],
                                        in1=mk_sb[:, :, :], op=mybir.AluOpType.mult)
            for hc in range(2):
                for b in range(Bc):
                    nc.vector.tensor_tensor_scan(
                        out=hs["2f"][:, :, hc, b],
                        data0=mk1_sb[:, :, b],
                        data1=hm[:, :, hc, b],
                        initial=0.0,
                        op0=mybir.AluOpType.mult,
                        op1=mybir.AluOpType.add)

            # ---- classifier
            with tc.tile_pool(name="cl", bufs=3) as cp, \
                 tc.tile_pool(name="cps", bufs=3, space="PSUM") as cps:
                for n in range(TB // 512):
                    psm = cps.tile([TAGS, 512], f32, tag="ps")
                    for k in range(4):
                        src = hs["2f"] if k < 2 else hs["2b"]
                        nc.tensor.matmul(
                            out=psm[:],
                            lhsT=cls_sb[:, k * TAGS:(k + 1) * TAGS],
                            rhs=src[:, n * 64:(n + 1) * 64, k % 2, :],
                            start=(k == 0), stop=(k == 3))
                    lg = cp.tile([TAGS, 512], f32, tag="lg")
                    nc.vector.tensor_scalar_add(lg[:], psm[:], clsb_sb[:, :1])
                    nc.sync.dma_start(logits_d[:, n * 512:(n + 1) * 512], lg[:])

    nc.compile()
    return nc


def kernel(**inputs):
    words = np.asarray(inputs["words"]).astype(np.int32)      # [B, T]
    lengths = np.asarray(inputs["lengths"]).astype(np.int32)  # [B]
    emb = np.asarray(inputs["emb"], dtype=np.float32)

    if "nc" not in _CACHE:
        _CACHE["nc"] = _build_program()
    nc = _CACHE["nc"]

    mask = (lengths[:, None] > np.arange(T)[None, :]).astype(np.float32)  # [B,T]
    wprep = {u: _prep_unit_weights(inputs[f"l{u}_Wih"], inputs[f"l{u}_Whh"],
                                   inputs[f"l{u}_bih"], inputs[f"l{u}_bhh"])
             for u in UNITS}
    clsW = np.asarray(inputs["cls_W"], dtype=np.float32)      # [50, 512]
    CT = clsW.T  # [512, 50]
    clsx = np.concatenate([CT[k * 128:(k + 1) * 128, :] for k in range(4)],
                          axis=1).astype(ml_dtypes.bfloat16)  # [128, 200]
    clsb = np.asarray(inputs["cls_b"], dtype=np.float32).reshape(TAGS, 1)

    in_maps = []
    for c in range(NCORES):
        bsl = slice(c * Bc, (c + 1) * Bc)
        w_c = words[bsl]                      # [Bc, T]
        m_c = mask[bsl]                       # [Bc, T]
        words_tm = np.ascontiguousarray(
            np.ascontiguousarray(w_c.T).reshape(TB // 128, 128).T)  # [128, 32]
        aug = np.stack([(1.0 - m_c.T).reshape(TB), np.ones(TB, np.float32)]
                       ).astype(ml_dtypes.bfloat16)           # [2, TB]
        mT = m_c.T.astype(ml_dtypes.bfloat16)                 # [T, Bc]
        mk = np.ascontiguousarray(np.broadcast_to(mT[None], (128, T, Bc)))
        mk1 = np.ascontiguousarray(
            np.broadcast_to((1.0 - m_c.T).astype(ml_dtypes.bfloat16)[None],
                            (128, T, Bc)))
        im = {"emb": emb, "words": words_tm, "aug": aug,
              "mk": mk, "mk1": mk1, "clsx": clsx, "clsb": clsb}
        for u in UNITS:
            wx, wa, wh = wprep[u]
            im[f"w{u}x"] = wx
            im[f"w{u}a"] = wa
            im[f"w{u}h"] = wh
        in_maps.append(im)

    _CACHE["in_maps"] = in_maps
    res = run_bass_kernel_spmd(nc, in_maps, list(range(NCORES)))
    out = np.empty((B, T, TAGS), np.float32)
    for c in range(NCORES):
        lg = res.results[c]["logits"]          # [50, TB], col = t*Bc + b
        out[c * Bc:(c + 1) * Bc] = lg.reshape(TAGS, T, Bc).transpose(2, 1, 0)
    return out


def bench(inputs):
    """Run once with NTFF tracing; returns HW exec_time_ns (and stashes trace)."""
    kernel(**inputs)  # ensure program built/cached
    nc = _CACHE["nc"]
    in_maps = _CACHE["in_maps"]
    import tempfile
    tmpdir = tempfile.mkdtemp(prefix="bilstm_trace_")
    res = run_bass_kernel_spmd(nc, in_maps, list(range(NCORES)), trace=True,
                               tmpdir=tmpdir)
    _CACHE["trace_dir"] = tmpdir
    _CACHE["last_bench"] = res
    print("trace dir:", tmpdir)
    return res.exec_time_ns


if __name__ == "__main__":
    import reference
    inputs = {k: np.asarray(v) for k, v in reference.setup_inputs().items()}
    got = kernel(**inputs)
    print(got.shape, got.dtype)


# revision 13
# speedup vs baseline: 2.4236x; 1.2198x over previous
"""BiLSTM tagger on 8 TRN2 NeuronCores.

Strategy (hardcoded for B=64,T=512,V=30000,E=128,H=256,TAGS=50):
  - Data-parallel: batch sharded 8 ways (8 sequences/core); weights replicated.
  - Per core: embedding gather (indirect DMA) -> PE transpose -> x^T in SBUF;
    input projections xg = W_ih_aug @ [x; 1-m; 1] precomputed for all t as big
    matmuls into DRAM scratch (bf16); recurrences as dynamic Tile loops with
    the two directions of a layer FUSED: one pointwise instruction covers both
    units ([128, 2u, chunk, Bc] APs), halving DVE/Act instruction count and
    keeping a single dependency chain.
  - Masking: (1-m) feature adds +/-60 to f/i gate pre-activations at masked
    steps, freezing c exactly. Backward h is then exactly 0 at masked steps.
    Forward l2 h at masked steps is repaired AFTER the recurrence with a
    tensor_tensor_scan per (hchunk, batch): hold[t] = (1-m)*hold[t-1]+m*h[t].
  - tanh(g) is computed as 2*sigmoid(2g)-1 by pre-doubling the g-gate rows:
    ONE sigmoid activation covers all 8 gate chunks. The cell state is
    carried as c/2 so the correction folds into existing multiplies
    (scalar_tensor_tensor) and tanh(c) = Tanh with scale=2.
  - Gate layout: gates on partitions (8 chunks of 128 = [i0 i1 f0 f1 o0 o1 g0
    g1]), batch on free dim; Whh stationary [128h x 128gate] bf16 tiles, h
    moving [128, 8]. Reverse-direction steps share the fused ops via custom
    APs whose unit-dim stride jumps to the mirrored column slot.
"""
import sys

sys.path.insert(0, "/opt/trn_rl_repo")
import contextlib

import numpy as np
import ml_dtypes

import concourse.bass as bass
import concourse.bacc as bacc
import concourse.mybir as mybir
import concourse.tile as tile
from concourse.ap import AP
from concourse.bass import ds
from concourse.bass_utils import run_bass_kernel_spmd
from concourse.masks import make_identity

B, T, V, E, H, TAGS = 64, 512, 30000, 128, 256, 50
NCORES = 8
Bc = B // NCORES          # 8 sequences per core
TB = T * Bc               # 4096 tokens per core
SPB = 32                  # steps per For_i body
NBODY = T // SPB          # 16
CPB = SPB * Bc            # xg columns per body (256)

f32 = mybir.dt.float32
bf16 = mybir.dt.bfloat16
i32 = mybir.dt.int32

UNITS = ("1f", "1b", "2f", "2b")
KCNT = {"1f": 1, "1b": 1, "2f": 4, "2b": 4}       # 128-row K chunks of x features
REV = {"1f": False, "1b": True, "2f": False, "2b": True}

# gate chunk order i0 i1 f0 f1 o0 o1 g0 g1 (torch row order is i f g o)
PERM = np.concatenate([np.arange(0, 256), np.arange(256, 512),
                       np.arange(768, 1024), np.arange(512, 768)])

_CACHE = {}


def _prep_unit_weights(Wih, Whh, bih, bhh):
    """Host-side weight marshalling for one LSTM direction (8 gate chunks).
    g-gate rows are doubled so tanh(g) = 2*sigmoid(2g)-1 on device."""
    din = Wih.shape[1]
    Wp = np.asarray(Wih)[PERM].astype(np.float64)   # [1024, din]
    Up = np.asarray(Whh)[PERM].astype(np.float64)   # [1024, 256]
    bp = (np.asarray(bih) + np.asarray(bhh)).astype(np.float64)[PERM]  # [1024]
    Wp[768:1024] *= 2.0
    Up[768:1024] *= 2.0
    bp[768:1024] *= 2.0
    M = 1024
    k_cnt = din // 128
    WT = Wp.T.astype(np.float32)        # [din, 1024]
    wx = np.concatenate([WT[k * 128:(k + 1) * 128, :] for k in range(k_cnt)],
                        axis=1).astype(ml_dtypes.bfloat16)  # [128, k_cnt*1024]
    # aug lhsT rows: feature0 = (1-m), feature1 = 1
    wa = np.zeros((2, M), np.float32)
    wa[0, 0:256] = -60.0   # i rows: -60*(1-m)
    wa[0, 256:512] = 60.0  # f rows: +60*(1-m)
    wa[1, :] = bp
    wa = wa.astype(ml_dtypes.bfloat16)
    UT = Up.T.astype(np.float32)
    wh = np.concatenate([UT[0:128, :], UT[128:256, :]], axis=1).astype(ml_dtypes.bfloat16)
    return wx, wa, wh


def _uap(base, us, ustride_delta, step):
    """AP over `base` (sliced to one step of the fwd unit) whose unit-dim
    stride is adjusted so unit 1 lands on the mirrored step slot."""
    pairs = [list(p) for p in base.ap]
    pairs[1][0] += ustride_delta
    return AP(tensor=base.tensor, offset=base.offset + us * step, ap=pairs)


def _build_program():
    nc = bacc.Bacc("TRN2", target_bir_lowering=False, debug=False, num_devices=NCORES)
    emb_d = nc.dram_tensor("emb", [V, E], f32, kind="ExternalInput")
    words_d = nc.dram_tensor("words", [128, TB // 128], i32, kind="ExternalInput")
    aug_d = nc.dram_tensor("aug", [2, TB], bf16, kind="ExternalInput")
    mk_d = nc.dram_tensor("mk", [128, T, Bc], bf16, kind="ExternalInput")
    mk1_d = nc.dram_tensor("mk1", [128, T, Bc], bf16, kind="ExternalInput")
    wxd, wad, whd, xgd = {}, {}, {}, {}
    for u in UNITS:
        wxd[u] = nc.dram_tensor(f"w{u}x", [128, KCNT[u] * 1024], bf16, kind="ExternalInput")
        wad[u] = nc.dram_tensor(f"w{u}a", [2, 1024], bf16, kind="ExternalInput")
        whd[u] = nc.dram_tensor(f"w{u}h", [128, 2048], bf16, kind="ExternalInput")
        xgd[u] = nc.dram_tensor(f"xg{u}", [8, 128, TB], bf16)
    clsx_d = nc.dram_tensor("clsx", [128, 4 * TAGS], bf16, kind="ExternalInput")
    clsb_d = nc.dram_tensor("clsb", [TAGS, 1], f32, kind="ExternalInput")
    logits_d = nc.dram_tensor("logits", [TAGS, TB], f32, kind="ExternalOutput")

    ctx = contextlib.ExitStack()
    with tile.TileContext(nc) as tc, ctx:
        pp = ctx.enter_context(tc.tile_pool(name="persist", bufs=1))
        xT = pp.tile([128, TB], bf16, tag="xT")
        aug_sb = pp.tile([2, TB], bf16, tag="aug")
        mk_sb = pp.tile([128, T, Bc], bf16, tag="mk")
        mk1_sb = pp.tile([128, T, Bc], bf16, tag="mk1")
        ident = pp.tile([128, 128], f32, tag="ident")
        wx_sb = {u: pp.tile([128, KCNT[u] * 1024], bf16, tag=f"wx{u}", name=f"wx{u}") for u in UNITS}
        wa_sb = {u: pp.tile([2, 1024], bf16, tag=f"wa{u}", name=f"wa{u}") for u in UNITS}
        wh_sb = {u: pp.tile([128, 2048], bf16, tag=f"wh{u}", name=f"wh{u}") for u in UNITS}
        cls_sb = pp.tile([128, 4 * TAGS], bf16, tag="clsx")
        clsb_sb = pp.tile([TAGS, 1], f32, tag="clsb")
        hs = {u: pp.tile([128, T, 2, Bc], bf16, tag=f"hs{u}", name=f"hs{u}") for u in UNITS}
        hcur = pp.tile([128, 2, 2, Bc], bf16, tag="hcur")   # [p, unit, hchunk, b]
        ccar = pp.tile([128, 2, 2, Bc], f32, tag="ccar")    # carries c/2
        identb = pp.tile([128, 128], bf16, tag="identb")

        # ---- load weights / constants
        make_identity(nc, ident[:])
        make_identity(nc, identb[:])
        for u in UNITS:
            nc.sync.dma_start(wx_sb[u][:], wxd[u][:])
            nc.sync.dma_start(wa_sb[u][:], wad[u][:])
            nc.sync.dma_start(wh_sb[u][:], whd[u][:])
        nc.sync.dma_start(cls_sb[:], clsx_d[:])
        nc.sync.dma_start(clsb_sb[:], clsb_d[:])
        nc.sync.dma_start(aug_sb[:], aug_d[:])
        nc.sync.dma_start(mk_sb[:], mk_d[:])
        nc.sync.dma_start(mk1_sb[:], mk1_d[:])

        # ---- embedding gather + transpose into xT
        with tc.tile_pool(name="gat", bufs=3) as gp, \
             tc.tile_pool(name="gps", bufs=3, space="PSUM") as gps:
            idx_all = gp.tile([128, TB // 128], i32, tag="idxall")
            nc.sync.dma_start(idx_all[:], words_d[:])
            for n in range(TB // 128):
                xt = gp.tile([128, 128], f32, tag="xt")
                nc.gpsimd.indirect_dma_start(
                    out=xt[:], out_offset=None, in_=emb_d[:, :],
                    in_offset=bass.IndirectOffsetOnAxis(ap=idx_all[:, n:n + 1], axis=0))
                pst = gps.tile([128, 128], f32, tag="pst")
                nc.tensor.transpose(out=pst[:], in_=xt[:], identity=ident[:])
                nc.vector.tensor_copy(xT[:, n * 128:(n + 1) * 128], pst[:])

        # ---- xg precompute: xg[u][m] = Wx_m^T x + Wa_m^T aug, all t
        # 4 psum banks per group of 4 m-chunks; one staging copy + one DMA per group
        def xg_precompute(u, rhs_of_k):
            k_cnt = KCNT[u]
            with tc.tile_pool(name=f"xp{u}", bufs=2, space="PSUM") as xps, \
                 tc.tile_pool(name=f"xs{u}", bufs=2) as xsb:
                for n in range(TB // 512):
                    nsl = slice(n * 512, (n + 1) * 512)
                    for mg in range(2):
                        psm = xps.tile([128, 4, 512], f32, tag="ps")
                        for mi in range(4):
                            m = mg * 4 + mi
                            for k in range(k_cnt):
                                nc.tensor.matmul(
                                    out=psm[:, mi, :],
                                    lhsT=wx_sb[u][:, (k * 8 + m) * 128:(k * 8 + m + 1) * 128],
                                    rhs=rhs_of_k(k, n),
                                    start=(k == 0), stop=False)
                            nc.tensor.matmul(
                                out=psm[:, mi, :],
                                lhsT=wa_sb[u][:, m * 128:(m + 1) * 128],
                                rhs=aug_sb[:, nsl],
                                start=False, stop=True)
                        stg = xsb.tile([128, 4, 512], bf16, tag="stg")
                        if mg == 0:
                            nc.vector.tensor_copy(stg[:], psm[:])
                        else:
                            nc.scalar.activation(stg[:], psm[:],
                                                 mybir.ActivationFunctionType.Copy)
                        nc.sync.dma_start(
                            xgd[u][mg * 4:(mg + 1) * 4, :, nsl].transpose([1, 0, 2]),
                            stg[:])

        def l1_rhs(k, n):
            return xT[:, n * 512:(n + 1) * 512]

        xg_precompute("1f", l1_rhs)
        xg_precompute("1b", l1_rhs)

        # ---- fused recurrence for a (fwd, rev) unit pair
        def phase(units):
            uF, uR = units
            nc.vector.memset(hcur[:, :, :, :], 0.0)
            nc.vector.memset(ccar[:, :, :, :], 0.0)
            with tc.tile_pool(name=f"rc{uF}", bufs=2) as rp, \
                 tc.tile_pool(name=f"rps{uF}", bufs=3, space="PSUM") as rps, \
                 tc.tile_pool(name=f"rtmp{uF}", bufs=3) as tp:
                with tc.For_i(0, NBODY, hint_engines=(mybir.EngineType.PE,)) as i:
                    xb = rp.tile([128, 2, 8, CPB], bf16, tag="xb")
                    colF = i * CPB
                    colR = i * (-CPB) + (TB - CPB)
                    nc.sync.dma_start(xb[:, 0, :, :],
                                      xgd[uF][:, :, ds(colF, CPB)].transpose([1, 0, 2]))
                    nc.sync.dma_start(xb[:, 1, :, :],
                                      xgd[uR][:, :, ds(colR, CPB)].transpose([1, 0, 2]))
                    # hstage: [p, unit, slot, hchunk, b]; slot = local t index
                    hstage = rp.tile([128, 2, SPB, 2, Bc], bf16, tag="hst")
                    tF = i * SPB
                    tR = i * (-SPB) + (T - SPB)
                    for us in range(SPB):
                        # per-unit PSUM banks; the two units' chains run
                        # independently, unit B's matmul burst overlapping unit
                        # A's pointwise. One open accumulation group per bank
                        # (cross-bank concurrency is safe; padding fills a bank).
                        psA = rps.tile([128, 8, 64], f32, tag="gA")
                        psB = rps.tile([128, 8, 64], f32, tag="gB")
                        colF_ = us * Bc
                        colR_ = (SPB - 1 - us) * Bc
                        nc.tensor.matmul(
                            out=psA[:, :, 0:Bc], lhsT=identb[:, :],
                            rhs=xb[:, 0, :, colF_:colF_ + Bc],
                            start=True, stop=False, skip_group_check=True)
                        nc.tensor.matmul(
                            out=psB[:, :, 0:Bc], lhsT=identb[:, :],
                            rhs=xb[:, 1, :, colR_:colR_ + Bc],
                            start=True, stop=False, skip_group_check=True)
                        for ui, ps in ((0, psA), (1, psB)):
                            for m in range(8):
                                for k in range(2):
                                    nc.tensor.matmul(
                                        out=ps[:, m, 0:Bc],
                                        lhsT=wh_sb[units[ui]][:, (k * 8 + m) * 128:(k * 8 + m + 1) * 128],
                                        rhs=hcur[:, ui, k, :],
                                        start=False,
                                        stop=(m == 7 and k == 1),
                                        skip_group_check=True)
                        sgA = tp.tile([128, 8, Bc], f32, tag="sgA")
                        sgB = tp.tile([128, 8, Bc], f32, tag="sgB")
                        csf = tp.tile([128, 2, 2, Bc], f32, tag="csf")
                        t1 = tp.tile([128, 2, 2, Bc], f32, tag="t1")
                        tc2 = tp.tile([128, 2, 2, Bc], f32, tag="tc2")
                        # emission order == readiness order per engine queue
                        nc.scalar.activation(sgA[:, :, :], psA[:, :, 0:Bc],
                                             mybir.ActivationFunctionType.Sigmoid)
                        nc.scalar.activation(sgB[:, :, :], psB[:, :, 0:Bc],
                                             mybir.ActivationFunctionType.Sigmoid)
                        for ui, sg in ((0, sgA), (1, sgB)):
                            nc.vector.tensor_tensor(
                                out=csf[:, ui, :, :], in0=sg[:, 2:4, :],
                                in1=ccar[:, ui, :, :], op=mybir.AluOpType.mult)
                            nc.vector.scalar_tensor_tensor(
                                out=t1[:, ui, :, :], in0=sg[:, 6:8, :], scalar=0.5,
                                in1=sg[:, 0:2, :],
                                op0=mybir.AluOpType.subtract, op1=mybir.AluOpType.mult)
                            nc.vector.tensor_tensor(
                                out=ccar[:, ui, :, :], in0=csf[:, ui, :, :],
                                in1=t1[:, ui, :, :], op=mybir.AluOpType.add)
                        nc.scalar.activation(tc2[:, 0, :, :], ccar[:, 0, :, :],
                                             mybir.ActivationFunctionType.Tanh,
                                             scale=2.0)
                        nc.scalar.activation(tc2[:, 1, :, :], ccar[:, 1, :, :],
                                             mybir.ActivationFunctionType.Tanh,
                                             scale=2.0)
                        for ui, sg in ((0, sgA), (1, sgB)):
                            nc.vector.tensor_tensor(
                                out=hcur[:, ui, :, :], in0=sg[:, 4:6, :],
                                in1=tc2[:, ui, :, :], op=mybir.AluOpType.mult)
                            slot = us if ui == 0 else SPB - 1 - us
                            nc.gpsimd.tensor_copy(hstage[:, ui, slot, :, :],
                                                  hcur[:, ui, :, :])
                    nc.scalar.activation(hs[uF][:, ds(tF, SPB), :, :],
                                         hstage[:, 0, :, :, :],
                                         mybir.ActivationFunctionType.Copy)
                    nc.scalar.activation(hs[uR][:, ds(tR, SPB), :, :],
                                         hstage[:, 1, :, :, :],
                                         mybir.ActivationFunctionType.Copy)

        phase(("1f", "1b"))

        def l2_rhs(k, n):
            src = hs["1f"] if k < 2 else hs["1b"]
            return src[:, n * 64:(n + 1) * 64, k % 2, :]

        xg_precompute("2f", l2_rhs)
        xg_precompute("2b", l2_rhs)

        phase(("2f", "2b"))

        # ---- masked-h hold for the forward l2 output:
        # hold[t] = (1-m[t])*hold[t-1] + m[t]*h[t], done as scans in place.
        with tc.tile_pool(name="scan", bufs=1) as sp:
            hm = sp.tile([128, T, 2, Bc], bf16, tag="hm")
            for hc in range(2):
                nc.vector.tensor_tensor(out=hm[:, :, hc, :], in0=hs["2f"][:, :, hc, :],
                                        in1=mk_sb[:, :, :], op=mybir.AluOpType.mult)
            for hc in range(2):
                for b in range(Bc):
                    nc.vector.tensor_tensor_scan(
                        out=hs["2f"][:, :, hc, b],
                        data0=mk1_sb[:, :, b],
                        data1=hm[:, :, hc, b],
                        initial=0.0,
                        op0=mybir.AluOpType.mult,
                        op1=mybir.AluOpType.add)

            # ---- classifier
            with tc.tile_pool(name="cl", bufs=3) as cp, \
                 tc.tile_pool(name="cps", bufs=3, space="PSUM") as cps:
                for n in range(TB // 512):
                    psm = cps.tile([TAGS, 512], f32, tag="ps")
                    for k in range(4):
                        src = hs["2f"] if k < 2 else hs["2b"]
                        nc.tensor.matmul(
                            out=psm[:],
                            lhsT=cls_sb[:, k * TAGS:(k + 1) * TAGS],
                            rhs=src[:, n * 64:(n + 1) * 64, k % 2, :],
                            start=(k == 0), stop=(k == 3))
                    lg = cp.tile([TAGS, 512], f32, tag="lg")
                    nc.vector.tensor_scalar_add(lg[:], psm[:], clsb_sb[:, :1])
                    nc.sync.dma_start(logits_d[:, n * 512:(n + 1) * 512], lg[:])

    nc.compile()
    return nc


def kernel(**inputs):
    words = np.asarray(inputs["words"]).astype(np.int32)      # [B, T]
    lengths = np.asarray(inputs["lengths"]).astype(np.int32)  # [B]
    emb = np.asarray(inputs["emb"], dtype=np.float32)

    if "nc" not in _CACHE:
        _CACHE["nc"] = _build_program()
    nc = _CACHE["nc"]

    mask = (lengths[:, None] > np.arange(T)[None, :]).astype(np.float32)  # [B,T]
    wprep = {u: _prep_unit_weights(inputs[f"l{u}_Wih"], inputs[f"l{u}_Whh"],
                                   inputs[f"l{u}_bih"], inputs[f"l{u}_bhh"])
             for u in UNITS}
    clsW = np.asarray(inputs["cls_W"], dtype=np.float32)      # [50, 512]
    CT = clsW.T  # [512, 50]
    clsx = np.concatenate([CT[k * 128:(k + 1) * 128, :] for k in range(4)],
                          axis=1).astype(ml_dtypes.bfloat16)  # [128, 200]
    clsb = np.asarray(inputs["cls_b"], dtype=np.float32).reshape(TAGS, 1)

    in_maps = []
    for c in range(NCORES):
        bsl = slice(c * Bc, (c + 1) * Bc)
        w_c = words[bsl]                      # [Bc, T]
        m_c = mask[bsl]                       # [Bc, T]
        words_tm = np.ascontiguousarray(
            np.ascontiguousarray(w_c.T).reshape(TB // 128, 128).T)  # [128, 32]
        aug = np.stack([(1.0 - m_c.T).reshape(TB), np.ones(TB, np.float32)]
                       ).astype(ml_dtypes.bfloat16)           # [2, TB]
        mT = m_c.T.astype(ml_dtypes.bfloat16)                 # [T, Bc]
        mk = np.ascontiguousarray(np.broadcast_to(mT[None], (128, T, Bc)))
        mk1 = np.ascontiguousarray(
            np.broadcast_to((1.0 - m_c.T).astype(ml_dtypes.bfloat16)[None],
                            (128, T, Bc)))
        im = {"emb": emb, "words": words_tm, "aug": aug,
              "mk": mk, "mk1": mk1, "clsx": clsx, "clsb": clsb}
        for u in UNITS:
            wx, wa, wh = wprep[u]
            im[f"w{u}x"] = wx
            im[f"w{u}a"] = wa
            im[f"w{u}h"] = wh
        in_maps.append(im)

    _CACHE["in_maps"] = in_maps
    res = run_bass_kernel_spmd(nc, in_maps, list(range(NCORES)))
    out = np.empty((B, T, TAGS), np.float32)
    for c in range(NCORES):
        lg = res.results[c]["logits"]          # [50, TB], col = t*Bc + b
        out[c * Bc:(c + 1) * Bc] = lg.reshape(TAGS, T, Bc).transpose(2, 1, 0)
    return out


def bench(inputs):
    """Run once with NTFF tracing; returns HW exec_time_ns (and stashes trace)."""
    kernel(**inputs)  # ensure program built/cached
    nc = _CACHE["nc"]
    in_maps = _CACHE["in_maps"]
    import tempfile
    tmpdir = tempfile.mkdtemp(prefix="bilstm_trace_")
    res = run_bass_kernel_spmd(nc, in_maps, list(range(NCORES)), trace=True,
                               tmpdir=tmpdir)
    _CACHE["trace_dir"] = tmpdir
    _CACHE["last_bench"] = res
    print("trace dir:", tmpdir)
    return res.exec_time_ns


if __name__ == "__main__":
    import reference
    inputs = {k: np.asarray(v) for k, v in reference.setup_inputs().items()}
    got = kernel(**inputs)
    print(got.shape, got.dtype)
